# revision 11
# baseline (speedup 1.0000x reference)
"""GCN-5 message-passing kernel for Trainium2, 8-core SPMD Bass/Tile.

Strategy (graph-parallel per the sharding hint), v2:
  - batch is sorted, so graphs are contiguous node ranges.  Core c owns graphs
    [125c, 125(c+1)) and their nodes; edges are assigned to the core owning the
    dst node.  Pooling / layernorm / output head are fully local per core.
  - Everything on the edge path is fp16 (tolerance is 2e-2): z rows are stored
    padded to 128 fp16 (256B) so one gather descriptor moves a full row.
  - Per layer: z = h @ W per local tile on the TensorEngine (fp16, 1 cyc/row
    vs fp32's 4), AllGather (fp16, same bytes as the old f32x64), then batched
    `dma_gather` (InstDMAGatherAnt, ~1us fixed + 0.34ns/row) replaces the old
    per-128-edge indirect_dma_start (~1.2us each) -- descriptor generation on
    GPSIMD drops from ~7ms to ~0.6ms.
  - dma_gather indices are int16, so the 101376-row z table is bucketed into
    4 ranges of 2 cores (2*S <= 32767 rows); edges are grouped per
    (tile-chunk, bucket) with uniform-across-cores padded block counts (the
    SPMD program is shared, so counts are max'd over cores; pads carry idx=0
    and norm=0).  Calls are capped at 4096 idxs with single_packet=False
    (larger/packed calls wedge the SWDGE ring on this runtime).
  - Scatter-add per 128-dst tile stays a one-hot matmul (fp16) with PSUM
    accumulation; the one-hot is built per chunk in one DVE is_equal.
  - Layer 5 aggregates in [node, H] orientation and feeds the per-graph
    mean-pool one-hot matmul directly, accumulating all tiles in one PSUM
    bank; layernorm + head are unchanged f32.
"""
import sys
import types
import contextlib

import numpy as np

sys.path.insert(0, "/opt/trn_rl_repo")

import concourse.bass as bass
import concourse.tile as tile
from concourse import mybir, library_config
from concourse.vector_clock import ScopedClock

F32 = mybir.dt.float32
F16 = mybir.dt.float16
I16 = mybir.dt.int16
M = 8  # NeuronCores
H = 64
CH = 8          # tiles per gather chunk
CALL_MAX = 4096  # max idxs per dma_gather call

# ---------------------------------------------------------------------------
# Environment fixes for this container
# ---------------------------------------------------------------------------

def _install_env_fixes():
    import concourse.tile as tile_mod

    def _patched_drain_and_barrier(self, tick_clock, wait_clock):
        # this walrus build allows a single sync-wait per TPB_CTRL Drain;
        # split the Tile tail-drain's waits across multiple drains.
        nc = self.nc
        drain_inst = nc.sync.drain()
        wait_clock.add_sem_waits(drain_inst.ins,
                                 ScopedClock({None: tick_clock.global_clock}))
        si = drain_inst.ins.sync_info
        waits = list(si.on_wait or [])
        if len(waits) > 1:
            si.on_wait[:] = waits[:1]
            for w in waits[1:]:
                d2 = nc.sync.drain()
                if d2.ins.sync_info is None:
                    d2.ins.sync_info = mybir.SyncInfo(on_wait=[w], on_update=[])
                else:
                    d2.ins.sync_info.on_wait.append(w)
        nc.all_engine_barrier()
        assert self.sems is not None
        popped = nc._tile_sem_poison_stack.pop()
        assert popped is self._sem_poison
        nc.clear_and_free_semaphores(list(self.sems.allocated().values()))
        nc.all_engine_barrier()

    tile_mod.TileContext._drain_and_barrier = _patched_drain_and_barrier

    _orig_lower_ordered = tile_mod.TileContext._lower_ordered_insts

    def _split_multiwait_lower(self, ordered):
        nc = self.nc
        for bbname, insts in ordered.items():
            newlist = []
            changed = False
            for inst in insts:
                si = getattr(inst, "sync_info", None)
                eng = getattr(inst, "engine", None)
                if (si is not None and si.on_wait and len(si.on_wait) > 1
                        and eng is not None and eng != mybir.EngineType.Unassigned
                        and inst.is_executable()):
                    waits = list(si.on_wait)
                    si.on_wait[:] = waits[-1:]
                    for w in waits[:-1]:
                        nop = mybir.InstNoOp(
                            name=nc.get_next_instruction_name(), engine=eng)
                        nop.sync_info = mybir.SyncInfo(on_wait=[w], on_update=[])
                        try:
                            nc.register_instruction(nop, overwrite=True)
                        except Exception:
                            pass
                        newlist.append(nop)
                    changed = True
                newlist.append(inst)
            if changed:
                insts[:] = newlist
        return _orig_lower_ordered(self, ordered)

    if getattr(tile_mod.TileContext._lower_ordered_insts, "__name__", "") != \
            "_split_multiwait_lower":
        tile_mod.TileContext._lower_ordered_insts = _split_multiwait_lower

    # NTFF profile hook (lets trace=True work under axon); best-effort.
    if "antenv.axon_hooks" not in sys.modules:
        try:
            from trn_agent_boot.trn_boot import _ntff_profile_via_ctypes
            hook = _ntff_profile_via_ctypes("/opt/axon/libaxon_pjrt.so")
            mod = types.ModuleType("antenv.axon_hooks")
            mod.get_axon_ntff_profile_hook = lambda: hook
            mod.set_axon_ntff_profile_hook = lambda h: None
            sys.modules["antenv.axon_hooks"] = mod
            import antenv
            antenv.axon_hooks = mod
        except Exception:
            pass


_install_env_fixes()

# ---------------------------------------------------------------------------
# Host preprocessing
# ---------------------------------------------------------------------------

def preprocess(x, edge_index, batch, n_graphs):
    N = x.shape[0]
    GPC = n_graphs // M
    e0 = np.asarray(edge_index[0], np.int64)
    e1 = np.asarray(edge_index[1], np.int64)
    loops = np.arange(N, dtype=np.int64)
    src = np.concatenate([e0, loops])
    dst = np.concatenate([e1, loops])
    deg = np.bincount(dst, minlength=N).astype(np.float32)
    dis = 1.0 / np.sqrt(deg)
    norm = (dis[src] * dis[dst]).astype(np.float32)

    batch = np.asarray(batch, np.int64)
    owner = batch // GPC
    node_start = np.searchsorted(batch, np.arange(M) * GPC)
    node_end = np.searchsorted(batch, np.arange(M) * GPC + GPC)
    n_c = node_end - node_start
    T = int(np.ceil(n_c.max() / 128))
    S = T * 128
    assert 2 * S <= 32767, (T, S)
    local = np.arange(N) - node_start[owner]
    row = owner * S + local

    eo = owner[dst]
    ld = local[dst]
    et = ld // 128
    ep = (ld % 128).astype(np.float32)
    er = row[src]
    eb = er // (2 * S)
    erel = er - eb * (2 * S)

    # uniform (max-over-cores) padded block counts per (tile, bucket)
    key = ((eo * T + et) * 4 + eb)
    counts = np.bincount(key, minlength=M * T * 4).reshape(M, T, 4)
    nblk_tb = -(-counts.max(axis=0) // 128)  # [T, 4]

    nchunks = -(-T // CH)
    blk_of_tb = np.zeros((T, 4), np.int64)
    chunk_info = []
    call_tb = []  # (t, b) per call, in program order, for the count tensor
    gb = 0
    for k in range(nchunks):
        t0, t1 = k * CH, min((k + 1) * CH, T)
        blk0 = gb
        calls = []
        for b in range(4):
            for t in range(t0, t1):
                blk_of_tb[t, b] = gb
                nb = int(nblk_tb[t, b])
                if nb > 0:
                    # one call per (tile, bucket) group: its -1 pads are
                    # trailing, so the per-core valid-count register lets the
                    # ucode generate descriptors only for real edges
                    calls.append((b, gb * 128, nb * 128, gb - blk0,
                                  len(call_tb)))
                    call_tb.append((t, b))
                gb += nb
        tiles = []
        for t in range(t0, t1):
            cols = []
            for b in range(4):
                st = int(blk_of_tb[t, b] - blk0)
                cols.extend(range(st, st + int(nblk_tb[t, b])))
            tiles.append((t, cols))
        chunk_info.append(dict(blk0=blk0, nblk=gb - blk0, calls=calls,
                               tiles=tiles))
    total_blocks = gb
    total_slots = gb * 128
    ncalls = len(call_tb)

    # per-core slot arrays
    ordk = np.argsort(key, kind="stable")
    ks = key[ordk]
    group_start = np.zeros(M * T * 4 + 1, np.int64)
    np.cumsum(counts.reshape(-1), out=group_start[1:])
    pos = np.arange(len(ks)) - group_start[ks]
    slot_base = blk_of_tb.reshape(-1)
    slot = slot_base[ks % (T * 4)] * 128 + pos
    core_of = ks // (T * 4)

    er_s = erel[ordk]
    nm_s = norm[ordk]
    ep_s = ep[ordk]

    cnt_g = np.bincount(batch, minlength=n_graphs).astype(np.float32)

    in_maps = []
    for c in range(M):
        selc = core_of == c
        sl = slot[selc]
        idx16 = np.full(total_slots, -1, np.int16)
        nrm = np.zeros(total_slots, np.float16)
        lds = np.zeros(total_slots, np.float16)
        idx16[sl] = er_s[selc].astype(np.int16)
        nrm[sl] = nm_s[selc].astype(np.float16)
        lds[sl] = ep_s[selc].astype(np.float16)
        # per-call valid counts for this core; empty groups get one dummy
        # valid idx so the DMA completion semaphore still fires
        cnts = np.zeros(ncalls, np.int32)
        for i, (t, b) in enumerate(call_tb):
            n = int(counts[c, t, b])
            if n == 0:
                idx16[blk_of_tb[t, b] * 128] = 0
                n = 1
            cnts[i] = n
        idxw = np.ascontiguousarray(
            np.tile(idx16.reshape(-1, 16).T, (8, 1)))
        nrm2 = np.ascontiguousarray(nrm.reshape(-1, 128).T)
        lds2 = np.ascontiguousarray(lds.reshape(-1, 128).T)

        xT = np.zeros((128, S), np.float16)
        xT[:, :n_c[c]] = x[node_start[c]:node_end[c]].T

        gflat = np.full(T * 128, -1.0, np.float16)
        gflat[:n_c[c]] = (batch[node_start[c]:node_end[c]] - c * GPC)
        gcol = np.ascontiguousarray(gflat.reshape(T, 128).T)

        invcnt = np.zeros((128, 1), np.float32)
        invcnt[:GPC, 0] = 1.0 / np.maximum(cnt_g[c * GPC:(c + 1) * GPC], 1.0)

        in_maps.append(dict(idx=idxw, nrm=nrm2, lds=lds2, xT=xT, gcol=gcol,
                            invcnt=invcnt, cnt=cnts.reshape(1, -1)))
    meta = dict(T=T, S=S, GPC=GPC, chunks=chunk_info,
                total_blocks=total_blocks, total_slots=total_slots,
                ncalls=ncalls)
    return in_maps, meta


def make_weight_inputs(W1, b1, Wh, bh, Wout, bout):
    Wh = np.asarray(Wh, np.float32)
    iota16 = np.broadcast_to(np.arange(128, dtype=np.float16), (128, 128)).copy()
    ident = np.eye(H, dtype=np.float32)
    return dict(
        W1=np.ascontiguousarray(W1, np.float32).astype(np.float16),
        Wh2=np.ascontiguousarray(
            Wh.transpose(1, 0, 2).reshape(H, 4 * H)).astype(np.float16),
        b14=np.ascontiguousarray(
            np.stack([b1, bh[0], bh[1], bh[2]], axis=1), np.float32),
        b5rep=np.broadcast_to(np.asarray(bh[3], np.float32), (128, H)).copy(),
        woutrep=np.broadcast_to(np.asarray(Wout, np.float32)[:, 0], (128, H)).copy(),
        iota16=iota16,
        ident=ident,
    ), dict(bout=np.asarray(bout, np.float32))


# ---------------------------------------------------------------------------
# Bass program
# ---------------------------------------------------------------------------

def build_nc(meta, weights):
    T, S = meta["T"], meta["S"]
    chunks = meta["chunks"]
    total_blocks = meta["total_blocks"]
    total_slots = meta["total_slots"]
    ncalls = meta["ncalls"]
    nc = bass.Bass("TRN2", target_bir_lowering=False)

    xT_d = nc.declare_dram_parameter("xT", [128, S], F16, isOutput=False)
    idx_d = nc.declare_dram_parameter("idx", [128, total_slots // 16], I16,
                                      isOutput=False)
    nrm_d = nc.declare_dram_parameter("nrm", [128, total_blocks], F16,
                                      isOutput=False)
    lds_d = nc.declare_dram_parameter("lds", [128, total_blocks], F16,
                                      isOutput=False)
    gcol_d = nc.declare_dram_parameter("gcol", [128, T], F16, isOutput=False)
    invcnt_d = nc.declare_dram_parameter("invcnt", [128, 1], F32, isOutput=False)
    iota16_d = nc.declare_dram_parameter("iota16", [128, 128], F16, isOutput=False)
    W1_d = nc.declare_dram_parameter("W1", [128, H], F16, isOutput=False)
    Wh2_d = nc.declare_dram_parameter("Wh2", [H, 4 * H], F16, isOutput=False)
    b14_d = nc.declare_dram_parameter("b14", [H, 4], F32, isOutput=False)
    b5rep_d = nc.declare_dram_parameter("b5rep", [128, H], F32, isOutput=False)
    woutrep_d = nc.declare_dram_parameter("woutrep", [128, H], F32, isOutput=False)
    ident_d = nc.declare_dram_parameter("ident", [H, H], F32, isOutput=False)
    cnt_d = nc.declare_dram_parameter("cnt", [1, ncalls], mybir.dt.int32,
                                      isOutput=False)
    out_d = nc.declare_dram_parameter("out", [128, 1], F32, isOutput=True)
    bout = float(weights["bout"][0])

    with tile.TileContext(nc) as tc:
        with contextlib.ExitStack() as ctx:
            zfull_pool = ctx.enter_context(
                tc.tile_pool(name="zfull", bufs=1, space="DRAM"))
            dram = ctx.enter_context(tc.tile_pool(name="dram", bufs=1, space="DRAM"))
            const = ctx.enter_context(tc.tile_pool(name="const", bufs=1))
            xp = ctx.enter_context(tc.tile_pool(name="xp", bufs=3))
            gp = ctx.enter_context(tc.tile_pool(name="gp", bufs=2))
            mp = ctx.enter_context(tc.tile_pool(name="mp", bufs=2))
            hp = ctx.enter_context(tc.tile_pool(name="hp", bufs=3))
            zp = ctx.enter_context(tc.tile_pool(name="zp", bufs=3))
            pop = ctx.enter_context(tc.tile_pool(name="pop", bufs=3))
            ep = ctx.enter_context(tc.tile_pool(name="ep", bufs=1))
            ps_agg = ctx.enter_context(tc.tile_pool(name="ps_agg", bufs=2, space="PSUM"))
            ps_z = ctx.enter_context(tc.tile_pool(name="ps_z", bufs=2, space="PSUM"))
            ps_pool = ctx.enter_context(tc.tile_pool(name="ps_pool", bufs=1, space="PSUM"))

            nc.gpsimd.load_library(library_config.mlp)
            # one shared register for dma_gather's num_idxs (to_reg would
            # allocate a fresh register per call and exhaust the file)
            nidx_reg = nc.alloc_register(mybir.EngineType.Pool, "nidx_reg")

            zfulls = [zfull_pool.tile([M * S, 128], F16, addr_space="Shared",
                                      name=f"zfull{k}", tag=f"zfull{k}")
                      for k in range(5)]
            bounces = [dram.tile([S, 128], F16, name=f"bounce{k}", tag=f"bounce{k}")
                       for k in range(5)]

            idx_sb = const.tile([128, total_slots // 16], I16)
            nc.sync.dma_start(idx_sb[:], idx_d[:])
            nrm_sb = const.tile([128, total_blocks], F16)
            nc.sync.dma_start(nrm_sb[:], nrm_d[:])
            lds_sb = const.tile([128, total_blocks], F16)
            nc.sync.dma_start(lds_sb[:], lds_d[:])
            gcol_sb = const.tile([128, T], F16)
            nc.sync.dma_start(gcol_sb[:], gcol_d[:])
            invcnt_sb = const.tile([128, 1], F32)
            nc.sync.dma_start(invcnt_sb[:], invcnt_d[:])
            iota16_sb = const.tile([128, 128], F16)
            nc.sync.dma_start(iota16_sb[:], iota16_d[:])
            W1_sb = const.tile([128, H], F16)
            nc.sync.dma_start(W1_sb[:], W1_d[:])
            Wh2_sb = const.tile([H, 4 * H], F16)
            nc.sync.dma_start(Wh2_sb[:], Wh2_d[:])
            b14_sb = const.tile([H, 4], F32)
            nc.sync.dma_start(b14_sb[:], b14_d[:])
            b5rep_sb = const.tile([128, H], F32)
            nc.sync.dma_start(b5rep_sb[:], b5rep_d[:])
            woutrep_sb = const.tile([128, H], F32)
            nc.sync.dma_start(woutrep_sb[:], woutrep_d[:])
            ident_sb = const.tile([H, H], F32)
            nc.sync.dma_start(ident_sb[:], ident_d[:])
            cnt_sb = const.tile([1, ncalls], mybir.dt.int32)
            nc.sync.dma_start(cnt_sb[:], cnt_d[:])

            # layer 0: z0 = x @ W1 per local tile, then AllGather
            for t in range(T):
                xt = xp.tile([128, 128], F16, tag="xt")
                nc.sync.dma_start(xt[:], xT_d[:, t * 128:(t + 1) * 128])
                pz = ps_z.tile([128, H], F32, space="PSUM", tag="pz")
                nc.tensor.matmul(out=pz[:], lhsT=xt[:], rhs=W1_sb[:],
                                 start=True, stop=True)
                zt = zp.tile([128, 128], F16, tag="zt")
                nc.scalar.copy(zt[:, :H], pz[:])
                nc.sync.dma_start(bounces[0][t * 128:(t + 1) * 128, :], zt[:])
            nc.gpsimd.collective_compute(
                "AllGather", mybir.AluOpType.bypass,
                replica_groups=[list(range(M))],
                ins=[bounces[0][:]], outs=[zfulls[0][:]])

            n_pool = sum(1 for ci in chunks for (t, cols) in ci["tiles"] if cols)
            pool_ps = None
            pool_i = 0

            for layer in range(1, 6):
                zf = zfulls[layer - 1]
                if layer == 5:
                    pool_ps = ps_pool.tile([H, 128], F32, space="PSUM", tag="pool")
                for ci in chunks:
                    cb = ci["nblk"]
                    blk0 = ci["blk0"]
                    g = gp.tile([128, cb, 128], F16, tag="g")
                    if layer == 1:
                        # define every slot once: later layers' short gathers
                        # (per-core valid prefix) leave pad slots stale, and
                        # stale must be finite for the nrm=0 multiply
                        nc.vector.memset(g[:].rearrange("p a b -> p (a b)"), 0.0)
                    for (b, slot0, nidx, blkoff, call_i) in ci["calls"]:
                        nc.gpsimd.reg_load(nidx_reg,
                                           cnt_sb[0:1, call_i:call_i + 1])
                        nc.gpsimd.dma_gather(
                            g[:, blkoff:blkoff + nidx // 128, :],
                            zf[b * 2 * S:(b + 1) * 2 * S, :],
                            idx_sb[:, slot0 // 16:(slot0 + nidx) // 16],
                            nidx, nidx_reg, 128, single_packet=False)
                    gs = g[:, :, 0:H]
                    nrm3 = nrm_sb[:, blk0:blk0 + cb, None].to_broadcast([128, cb, H])
                    nc.vector.tensor_tensor(out=gs, in0=gs, in1=nrm3,
                                            op=mybir.AluOpType.mult)
                    m01 = mp.tile([128, cb, 128], F16, tag="m01")
                    iota3 = iota16_sb[:, None, :].to_broadcast([128, cb, 128])
                    lds3 = lds_sb[:, blk0:blk0 + cb, None].to_broadcast([128, cb, 128])
                    nc.vector.tensor_tensor(out=m01[:], in0=iota3, in1=lds3,
                                            op=mybir.AluOpType.is_equal)
                    for (t, cols) in ci["tiles"]:
                        if not cols:
                            continue
                        if layer < 5:
                            pT = ps_agg.tile([H, 128], F32, space="PSUM", tag="pT")
                            for j, col in enumerate(cols):
                                nc.tensor.matmul(
                                    out=pT[:], lhsT=g[:, col, 0:H],
                                    rhs=m01[:, col, :],
                                    start=(j == 0), stop=(j == len(cols) - 1))
                            hT = hp.tile([H, 128], F16, tag="hT")
                            nc.scalar.activation(
                                hT[:], pT[:], mybir.ActivationFunctionType.Relu,
                                bias=b14_sb[:, layer - 1:layer])
                            pz = ps_z.tile([128, H], F32, space="PSUM", tag="pz")
                            nc.tensor.matmul(
                                out=pz[:], lhsT=hT[:],
                                rhs=Wh2_sb[:, (layer - 1) * H:layer * H],
                                start=True, stop=True)
                            zt = zp.tile([128, 128], F16, tag="zt")
                            nc.scalar.copy(zt[:, :H], pz[:])
                            nc.sync.dma_start(
                                bounces[layer][t * 128:(t + 1) * 128, :], zt[:])
                        else:
                            p5 = ps_agg.tile([128, H], F32, space="PSUM", tag="pT")
                            for j, col in enumerate(cols):
                                nc.tensor.matmul(
                                    out=p5[:], lhsT=m01[:, col, :],
                                    rhs=g[:, col, 0:H],
                                    start=(j == 0), stop=(j == len(cols) - 1))
                            h5 = hp.tile([128, H], F16, tag="hT")
                            nc.vector.tensor_tensor(out=h5[:], in0=p5[:],
                                                    in1=b5rep_sb[:],
                                                    op=mybir.AluOpType.add)
                            nc.scalar.activation(h5[:], h5[:],
                                                 mybir.ActivationFunctionType.Relu)
                            pt = pop.tile([128, 128], F16, tag="pt")
                            nc.vector.tensor_tensor(
                                out=pt[:], in0=iota16_sb[:],
                                in1=gcol_sb[:, t:t + 1].to_broadcast([128, 128]),
                                op=mybir.AluOpType.is_equal)
                            nc.tensor.matmul(
                                out=pool_ps[:], lhsT=h5[:], rhs=pt[:],
                                start=(pool_i == 0), stop=(pool_i == n_pool - 1))
                            pool_i += 1
                if layer < 5:
                    nc.gpsimd.collective_compute(
                        "AllGather", mybir.AluOpType.bypass,
                        replica_groups=[list(range(M))],
                        ins=[bounces[layer][:]], outs=[zfulls[layer][:]])

            # transpose, scale by 1/cnt, layernorm, output head (f32 tail)
            pool_sb = ep.tile([H, 128], F32, tag="poolsb")
            nc.scalar.copy(pool_sb[:], pool_ps[:])
            ptr = ps_z.tile([128, H], F32, space="PSUM", tag="pz")
            nc.tensor.transpose(out=ptr[:], in_=pool_sb[:], identity=ident_sb[:])
            pooled = ep.tile([128, H], F32, tag="pooled")
            nc.vector.tensor_scalar(out=pooled[:], in0=ptr[:],
                                    scalar1=invcnt_sb[:, 0:1], scalar2=None,
                                    op0=mybir.AluOpType.mult)
            mu = ep.tile([128, 1], F32, tag="mu")
            nc.vector.tensor_reduce(out=mu[:], in_=pooled[:],
                                    axis=mybir.AxisListType.X,
                                    op=mybir.AluOpType.add)
            nc.vector.tensor_scalar(out=mu[:], in0=mu[:], scalar1=1.0 / H,
                                    scalar2=None, op0=mybir.AluOpType.mult)
            xc = ep.tile([128, H], F32, tag="xc")
            nc.vector.tensor_scalar(out=xc[:], in0=pooled[:], scalar1=mu[:, 0:1],
                                    scalar2=None, op0=mybir.AluOpType.subtract)
            sq = ep.tile([128, H], F32, tag="sq")
            nc.scalar.activation(sq[:], xc[:], mybir.ActivationFunctionType.Square)
            var = ep.tile([128, 1], F32, tag="var")
            nc.vector.tensor_reduce(out=var[:], in_=sq[:],
                                    axis=mybir.AxisListType.X,
                                    op=mybir.AluOpType.add)
            nc.vector.tensor_scalar(out=var[:], in0=var[:], scalar1=1.0 / H,
                                    scalar2=None, op0=mybir.AluOpType.mult)
            eps_col = ep.tile([128, 1], F32, tag="eps")
            nc.vector.memset(eps_col[:], 1e-5)
            std = ep.tile([128, 1], F32, tag="std")
            nc.scalar.activation(std[:], var[:], mybir.ActivationFunctionType.Sqrt,
                                 bias=eps_col[:, 0:1])
            rstd = ep.tile([128, 1], F32, tag="rstd")
            nc.vector.reciprocal(rstd[:], std[:])
            ln = ep.tile([128, H], F32, tag="ln")
            nc.vector.tensor_scalar(out=ln[:], in0=xc[:], scalar1=rstd[:, 0:1],
                                    scalar2=None, op0=mybir.AluOpType.mult)
            y = ep.tile([128, H], F32, tag="y")
            nc.vector.tensor_tensor(out=y[:], in0=ln[:], in1=woutrep_sb[:],
                                    op=mybir.AluOpType.mult)
            yr = ep.tile([128, 1], F32, tag="yr")
            nc.vector.tensor_reduce(out=yr[:], in_=y[:], axis=mybir.AxisListType.X,
                                    op=mybir.AluOpType.add)
            nc.vector.tensor_scalar(out=yr[:], in0=yr[:], scalar1=bout,
                                    scalar2=None, op0=mybir.AluOpType.add)
            nc.sync.dma_start(out_d[:], yr[:])

    from concourse.library_overlay import lower_extended_insts
    lower_extended_insts(nc)
    return nc


# ---------------------------------------------------------------------------
# Entry point
# ---------------------------------------------------------------------------

def kernel(x, edge_index, batch, W1, b1, Wh, bh, Wout, bout):
    from concourse.bass_utils import run_bass_kernel_spmd

    x = np.asarray(x, np.float32)
    edge_index = np.asarray(edge_index)
    batch = np.asarray(batch)
    n_graphs = 1000

    in_maps, meta = preprocess(x, edge_index, batch, n_graphs)
    wmaps, wmeta = make_weight_inputs(W1, b1, Wh, bh, Wout, bout)
    nc = build_nc(meta, dict(bout=wmeta["bout"]))
    for im in in_maps:
        im.update(wmaps)

    import time
    last_err = None
    for attempt in range(3):
        try:
            res = run_bass_kernel_spmd(nc, in_maps, core_ids=list(range(M)))
            break
        except Exception as e:  # transient terminal hiccups / device recovery
            last_err = e
            time.sleep(30 * (attempt + 1))
    else:
        raise last_err

    GPC = meta["GPC"]
    out = np.concatenate([res.results[c]["out"][:GPC] for c in range(M)], axis=0)
    return np.ascontiguousarray(out, np.float32)


# revision 17
# speedup vs baseline: 1.3047x; 1.3047x over previous
"""GCN-5 message-passing kernel for Trainium2, 8-core SPMD Bass/Tile.

Strategy (graph-parallel per the sharding hint), v2:
  - batch is sorted, so graphs are contiguous node ranges.  Core c owns graphs
    [125c, 125(c+1)) and their nodes; edges are assigned to the core owning the
    dst node.  Pooling / layernorm / output head are fully local per core.
  - Everything on the edge path is fp16 (tolerance is 2e-2): z rows are stored
    padded to 128 fp16 (256B) so one gather descriptor moves a full row.
  - Per layer: z = h @ W per local tile on the TensorEngine (fp16, 1 cyc/row
    vs fp32's 4), AllGather (fp16, same bytes as the old f32x64), then batched
    `dma_gather` (InstDMAGatherAnt, ~1us fixed + 0.34ns/row) replaces the old
    per-128-edge indirect_dma_start (~1.2us each) -- descriptor generation on
    GPSIMD drops from ~7ms to ~0.6ms.
  - dma_gather indices are int16, so the 101376-row z table is bucketed into
    4 ranges of 2 cores (2*S <= 32767 rows); edges are grouped per
    (tile-chunk, bucket) with uniform-across-cores padded block counts (the
    SPMD program is shared, so counts are max'd over cores; pads carry idx=0
    and norm=0).  Calls are capped at 4096 idxs with single_packet=False
    (larger/packed calls wedge the SWDGE ring on this runtime).
  - Scatter-add per 128-dst tile stays a one-hot matmul (fp16) with PSUM
    accumulation; the one-hot is built per chunk in one DVE is_equal.
  - Layer 5 aggregates in [node, H] orientation and feeds the per-graph
    mean-pool one-hot matmul directly, accumulating all tiles in one PSUM
    bank; layernorm + head are unchanged f32.
"""
import sys
import types
import contextlib

import numpy as np

sys.path.insert(0, "/opt/trn_rl_repo")

import concourse.bass as bass
import concourse.tile as tile
from concourse import mybir, library_config
from concourse.vector_clock import ScopedClock

F32 = mybir.dt.float32
F16 = mybir.dt.float16
I16 = mybir.dt.int16
M = 8  # NeuronCores
H = 64
CH = 8          # tiles per gather chunk
CALL_MAX = 4096  # max idxs per dma_gather call

# ---------------------------------------------------------------------------
# Environment fixes for this container
# ---------------------------------------------------------------------------

def _install_env_fixes():
    import concourse.tile as tile_mod

    def _patched_drain_and_barrier(self, tick_clock, wait_clock):
        # this walrus build allows a single sync-wait per TPB_CTRL Drain;
        # split the Tile tail-drain's waits across multiple drains.
        nc = self.nc
        drain_inst = nc.sync.drain()
        wait_clock.add_sem_waits(drain_inst.ins,
                                 ScopedClock({None: tick_clock.global_clock}))
        si = drain_inst.ins.sync_info
        waits = list(si.on_wait or [])
        if len(waits) > 1:
            si.on_wait[:] = waits[:1]
            for w in waits[1:]:
                d2 = nc.sync.drain()
                if d2.ins.sync_info is None:
                    d2.ins.sync_info = mybir.SyncInfo(on_wait=[w], on_update=[])
                else:
                    d2.ins.sync_info.on_wait.append(w)
        nc.all_engine_barrier()
        assert self.sems is not None
        popped = nc._tile_sem_poison_stack.pop()
        assert popped is self._sem_poison
        nc.clear_and_free_semaphores(list(self.sems.allocated().values()))
        nc.all_engine_barrier()

    tile_mod.TileContext._drain_and_barrier = _patched_drain_and_barrier

    _orig_lower_ordered = tile_mod.TileContext._lower_ordered_insts

    def _split_multiwait_lower(self, ordered):
        nc = self.nc
        for bbname, insts in ordered.items():
            newlist = []
            changed = False
            for inst in insts:
                si = getattr(inst, "sync_info", None)
                eng = getattr(inst, "engine", None)
                if (si is not None and si.on_wait and len(si.on_wait) > 1
                        and eng is not None and eng != mybir.EngineType.Unassigned
                        and inst.is_executable()):
                    waits = list(si.on_wait)
                    si.on_wait[:] = waits[-1:]
                    for w in waits[:-1]:
                        nop = mybir.InstNoOp(
                            name=nc.get_next_instruction_name(), engine=eng)
                        nop.sync_info = mybir.SyncInfo(on_wait=[w], on_update=[])
                        try:
                            nc.register_instruction(nop, overwrite=True)
                        except Exception:
                            pass
                        newlist.append(nop)
                    changed = True
                newlist.append(inst)
            if changed:
                insts[:] = newlist
        return _orig_lower_ordered(self, ordered)

    if getattr(tile_mod.TileContext._lower_ordered_insts, "__name__", "") != \
            "_split_multiwait_lower":
        tile_mod.TileContext._lower_ordered_insts = _split_multiwait_lower

    # NTFF profile hook (lets trace=True work under axon); best-effort.
    if "antenv.axon_hooks" not in sys.modules:
        try:
            from trn_agent_boot.trn_boot import _ntff_profile_via_ctypes
            hook = _ntff_profile_via_ctypes("/opt/axon/libaxon_pjrt.so")
            mod = types.ModuleType("antenv.axon_hooks")
            mod.get_axon_ntff_profile_hook = lambda: hook
            mod.set_axon_ntff_profile_hook = lambda h: None
            sys.modules["antenv.axon_hooks"] = mod
            import antenv
            antenv.axon_hooks = mod
        except Exception:
            pass


_install_env_fixes()

# ---------------------------------------------------------------------------
# Host preprocessing
# ---------------------------------------------------------------------------

def preprocess(x, edge_index, batch, n_graphs):
    N = x.shape[0]
    GPC = n_graphs // M
    e0 = np.asarray(edge_index[0], np.int64)
    e1 = np.asarray(edge_index[1], np.int64)
    loops = np.arange(N, dtype=np.int64)
    src = np.concatenate([e0, loops])
    dst = np.concatenate([e1, loops])
    deg = np.bincount(dst, minlength=N).astype(np.float32)
    dis = 1.0 / np.sqrt(deg)
    norm = (dis[src] * dis[dst]).astype(np.float32)

    batch = np.asarray(batch, np.int64)
    owner = batch // GPC
    node_start = np.searchsorted(batch, np.arange(M) * GPC)
    node_end = np.searchsorted(batch, np.arange(M) * GPC + GPC)
    n_c = node_end - node_start
    T = int(np.ceil(n_c.max() / 128))
    S = T * 128
    assert 2 * S <= 32767, (T, S)
    local = np.arange(N) - node_start[owner]
    row = owner * S + local

    eo = owner[dst]
    ld = local[dst]
    et = ld // 128
    ep = (ld % 128).astype(np.float32)
    er = row[src]
    eb = er // (2 * S)
    erel = er - eb * (2 * S)

    # uniform (max-over-cores) padded block counts per (tile, bucket)
    key = ((eo * T + et) * 4 + eb)
    counts = np.bincount(key, minlength=M * T * 4).reshape(M, T, 4)
    nblk_tb = -(-counts.max(axis=0) // 128)  # [T, 4]

    nchunks = -(-T // CH)
    blk_of_tb = np.zeros((T, 4), np.int64)
    chunk_info = []
    call_tb = []  # (t, b) per call, in program order, for the count tensor
    gb = 0
    for k in range(nchunks):
        t0, t1 = k * CH, min((k + 1) * CH, T)
        blk0 = gb
        calls = []
        for b in range(4):
            rblk0 = gb
            for t in range(t0, t1):
                blk_of_tb[t, b] = gb
                gb += int(nblk_tb[t, b])
            nreg = gb - rblk0
            off = 0
            while off < nreg:
                nb = min(nreg - off, CALL_MAX // 128)
                calls.append((b, (rblk0 + off) * 128, nb * 128,
                              (rblk0 + off) - blk0, len(call_tb)))
                call_tb.append(None)
                off += nb
        tiles = []
        for t in range(t0, t1):
            cols = []
            for b in range(4):
                st = int(blk_of_tb[t, b] - blk0)
                cols.extend(range(st, st + int(nblk_tb[t, b])))
            tiles.append((t, cols))
        chunk_info.append(dict(blk0=blk0, nblk=gb - blk0, calls=calls,
                               tiles=tiles))
    total_blocks = gb
    total_slots = gb * 128
    ncalls = len(call_tb)

    # per-core slot arrays
    ordk = np.argsort(key, kind="stable")
    ks = key[ordk]
    group_start = np.zeros(M * T * 4 + 1, np.int64)
    np.cumsum(counts.reshape(-1), out=group_start[1:])
    pos = np.arange(len(ks)) - group_start[ks]
    slot_base = blk_of_tb.reshape(-1)
    slot = slot_base[ks % (T * 4)] * 128 + pos
    core_of = ks // (T * 4)

    er_s = erel[ordk]
    nm_s = norm[ordk]
    ep_s = ep[ordk]

    cnt_g = np.bincount(batch, minlength=n_graphs).astype(np.float32)

    in_maps = []
    for c in range(M):
        selc = core_of == c
        sl = slot[selc]
        idx16 = np.zeros(total_slots, np.int16)
        nrm = np.zeros(total_slots, np.float16)
        lds = np.zeros(total_slots, np.float16)
        idx16[sl] = er_s[selc].astype(np.int16)
        nrm[sl] = nm_s[selc].astype(np.float16)
        lds[sl] = ep_s[selc].astype(np.float16)
        idxw = np.ascontiguousarray(
            np.tile(idx16.reshape(-1, 16).T, (8, 1)))
        nrm2 = np.ascontiguousarray(nrm.reshape(-1, 128).T)
        lds2 = np.ascontiguousarray(lds.reshape(-1, 128).T)

        xT = np.zeros((128, S), np.float16)
        xT[:, :n_c[c]] = x[node_start[c]:node_end[c]].T

        gflat = np.full(T * 128, -1.0, np.float16)
        gflat[:n_c[c]] = (batch[node_start[c]:node_end[c]] - c * GPC)
        gcol = np.ascontiguousarray(gflat.reshape(T, 128).T)

        invcnt = np.zeros((128, 1), np.float32)
        invcnt[:GPC, 0] = 1.0 / np.maximum(cnt_g[c * GPC:(c + 1) * GPC], 1.0)

        in_maps.append(dict(idx=idxw, nrm=nrm2, lds=lds2, xT=xT, gcol=gcol,
                            invcnt=invcnt))
    meta = dict(T=T, S=S, GPC=GPC, chunks=chunk_info,
                total_blocks=total_blocks, total_slots=total_slots,
                ncalls=ncalls)
    return in_maps, meta


def make_weight_inputs(W1, b1, Wh, bh, Wout, bout):
    Wh = np.asarray(Wh, np.float32)
    iota16 = np.broadcast_to(np.arange(128, dtype=np.float16), (128, 128)).copy()
    ident = np.eye(H, dtype=np.float32)
    return dict(
        W1=np.ascontiguousarray(W1, np.float32).astype(np.float16),
        Wh2=np.ascontiguousarray(
            Wh.transpose(1, 0, 2).reshape(H, 4 * H)).astype(np.float16),
        b14=np.ascontiguousarray(
            np.stack([b1, bh[0], bh[1], bh[2]], axis=1), np.float32),
        b5rep=np.broadcast_to(np.asarray(bh[3], np.float32), (128, H)).copy(),
        woutrep=np.broadcast_to(np.asarray(Wout, np.float32)[:, 0], (128, H)).copy(),
        iota16=iota16,
        ident=ident,
    ), dict(bout=np.asarray(bout, np.float32))


# ---------------------------------------------------------------------------
# Bass program
# ---------------------------------------------------------------------------

def build_nc(meta, weights):
    T, S = meta["T"], meta["S"]
    chunks = meta["chunks"]
    total_blocks = meta["total_blocks"]
    total_slots = meta["total_slots"]
    ncalls = meta["ncalls"]
    nc = bass.Bass("TRN2", target_bir_lowering=False)

    xT_d = nc.declare_dram_parameter("xT", [128, S], F16, isOutput=False)
    idx_d = nc.declare_dram_parameter("idx", [128, total_slots // 16], I16,
                                      isOutput=False)
    nrm_d = nc.declare_dram_parameter("nrm", [128, total_blocks], F16,
                                      isOutput=False)
    lds_d = nc.declare_dram_parameter("lds", [128, total_blocks], F16,
                                      isOutput=False)
    gcol_d = nc.declare_dram_parameter("gcol", [128, T], F16, isOutput=False)
    invcnt_d = nc.declare_dram_parameter("invcnt", [128, 1], F32, isOutput=False)
    iota16_d = nc.declare_dram_parameter("iota16", [128, 128], F16, isOutput=False)
    W1_d = nc.declare_dram_parameter("W1", [128, H], F16, isOutput=False)
    Wh2_d = nc.declare_dram_parameter("Wh2", [H, 4 * H], F16, isOutput=False)
    b14_d = nc.declare_dram_parameter("b14", [H, 4], F32, isOutput=False)
    b5rep_d = nc.declare_dram_parameter("b5rep", [128, H], F32, isOutput=False)
    woutrep_d = nc.declare_dram_parameter("woutrep", [128, H], F32, isOutput=False)
    ident_d = nc.declare_dram_parameter("ident", [H, H], F32, isOutput=False)
    out_d = nc.declare_dram_parameter("out", [128, 1], F32, isOutput=True)
    bout = float(weights["bout"][0])

    with tile.TileContext(nc) as tc:
        with contextlib.ExitStack() as ctx:
            zfull_pool = ctx.enter_context(
                tc.tile_pool(name="zfull", bufs=1, space="DRAM"))
            dram = ctx.enter_context(tc.tile_pool(name="dram", bufs=1, space="DRAM"))
            const = ctx.enter_context(tc.tile_pool(name="const", bufs=1))
            xp = ctx.enter_context(tc.tile_pool(name="xp", bufs=3))
            gp = ctx.enter_context(tc.tile_pool(name="gp", bufs=2))
            mp = ctx.enter_context(tc.tile_pool(name="mp", bufs=2))
            hp = ctx.enter_context(tc.tile_pool(name="hp", bufs=3))
            zp = ctx.enter_context(tc.tile_pool(name="zp", bufs=3))
            pop = ctx.enter_context(tc.tile_pool(name="pop", bufs=3))
            ep = ctx.enter_context(tc.tile_pool(name="ep", bufs=1))
            ps_agg = ctx.enter_context(tc.tile_pool(name="ps_agg", bufs=2, space="PSUM"))
            ps_z = ctx.enter_context(tc.tile_pool(name="ps_z", bufs=2, space="PSUM"))
            ps_pool = ctx.enter_context(tc.tile_pool(name="ps_pool", bufs=1, space="PSUM"))

            nc.gpsimd.load_library(library_config.mlp)
            # one shared register for dma_gather's num_idxs (to_reg would
            # allocate a fresh register per call and exhaust the file)
            nidx_reg = nc.alloc_register(mybir.EngineType.Pool, "nidx_reg")

            zfulls = [zfull_pool.tile([M * S, 128], F16, addr_space="Shared",
                                      name=f"zfull{k}", tag=f"zfull{k}")
                      for k in range(5)]
            bounces = [dram.tile([S, 128], F16, name=f"bounce{k}", tag=f"bounce{k}")
                       for k in range(5)]

            idx_sb = const.tile([128, total_slots // 16], I16)
            nc.sync.dma_start(idx_sb[:], idx_d[:])
            nrm_sb = const.tile([128, total_blocks], F16)
            nc.sync.dma_start(nrm_sb[:], nrm_d[:])
            lds_sb = const.tile([128, total_blocks], F16)
            nc.sync.dma_start(lds_sb[:], lds_d[:])
            gcol_sb = const.tile([128, T], F16)
            nc.sync.dma_start(gcol_sb[:], gcol_d[:])
            invcnt_sb = const.tile([128, 1], F32)
            nc.sync.dma_start(invcnt_sb[:], invcnt_d[:])
            iota16_sb = const.tile([128, 128], F16)
            nc.sync.dma_start(iota16_sb[:], iota16_d[:])
            W1_sb = const.tile([128, H], F16)
            nc.sync.dma_start(W1_sb[:], W1_d[:])
            Wh2_sb = const.tile([H, 4 * H], F16)
            nc.sync.dma_start(Wh2_sb[:], Wh2_d[:])
            b14_sb = const.tile([H, 4], F32)
            nc.sync.dma_start(b14_sb[:], b14_d[:])
            b5rep_sb = const.tile([128, H], F32)
            nc.sync.dma_start(b5rep_sb[:], b5rep_d[:])
            woutrep_sb = const.tile([128, H], F32)
            nc.sync.dma_start(woutrep_sb[:], woutrep_d[:])
            ident_sb = const.tile([H, H], F32)
            nc.sync.dma_start(ident_sb[:], ident_d[:])

            # layer 0: z0 = x @ W1 per local tile, then AllGather
            for t in range(T):
                xt = xp.tile([128, 128], F16, tag="xt")
                nc.sync.dma_start(xt[:], xT_d[:, t * 128:(t + 1) * 128])
                pz = ps_z.tile([128, H], F32, space="PSUM", tag="pz")
                nc.tensor.matmul(out=pz[:], lhsT=xt[:], rhs=W1_sb[:],
                                 start=True, stop=True)
                zt = zp.tile([128, 128], F16, tag="zt")
                nc.scalar.copy(zt[:, :H], pz[:])
                nc.sync.dma_start(bounces[0][t * 128:(t + 1) * 128, :], zt[:])
            nc.gpsimd.collective_compute(
                "AllGather", mybir.AluOpType.bypass,
                replica_groups=[list(range(M))],
                ins=[bounces[0][:]], outs=[zfulls[0][:]])

            n_pool = sum(1 for ci in chunks for (t, cols) in ci["tiles"] if cols)
            pool_ps = None
            pool_i = 0

            for layer in range(1, 6):
                zf = zfulls[layer - 1]
                if layer == 5:
                    pool_ps = ps_pool.tile([H, 128], F32, space="PSUM", tag="pool")
                for ci in chunks:
                    cb = ci["nblk"]
                    blk0 = ci["blk0"]
                    g = gp.tile([128, cb, 128], F16, tag="g")
                    for (b, slot0, nidx, blkoff, call_i) in ci["calls"]:
                        nc.gpsimd.reg_mov(nidx_reg, nidx)
                        nc.gpsimd.dma_gather(
                            g[:, blkoff:blkoff + nidx // 128, :],
                            zf[b * 2 * S:(b + 1) * 2 * S, :],
                            idx_sb[:, slot0 // 16:(slot0 + nidx) // 16],
                            nidx, nidx_reg, 128, single_packet=False)
                    gs = g[:, :, 0:H]
                    nrm3 = nrm_sb[:, blk0:blk0 + cb, None].to_broadcast([128, cb, H])
                    nc.vector.tensor_tensor(out=gs, in0=gs, in1=nrm3,
                                            op=mybir.AluOpType.mult)
                    m01 = mp.tile([128, cb, 128], F16, tag="m01")
                    iota3 = iota16_sb[:, None, :].to_broadcast([128, cb, 128])
                    lds3 = lds_sb[:, blk0:blk0 + cb, None].to_broadcast([128, cb, 128])
                    nc.vector.tensor_tensor(out=m01[:], in0=iota3, in1=lds3,
                                            op=mybir.AluOpType.is_equal)
                    for (t, cols) in ci["tiles"]:
                        if not cols:
                            continue
                        if layer < 5:
                            pT = ps_agg.tile([H, 128], F32, space="PSUM", tag="pT")
                            for j, col in enumerate(cols):
                                nc.tensor.matmul(
                                    out=pT[:], lhsT=g[:, col, 0:H],
                                    rhs=m01[:, col, :],
                                    start=(j == 0), stop=(j == len(cols) - 1))
                            hT = hp.tile([H, 128], F16, tag="hT")
                            nc.scalar.activation(
                                hT[:], pT[:], mybir.ActivationFunctionType.Relu,
                                bias=b14_sb[:, layer - 1:layer])
                            pz = ps_z.tile([128, H], F32, space="PSUM", tag="pz")
                            nc.tensor.matmul(
                                out=pz[:], lhsT=hT[:],
                                rhs=Wh2_sb[:, (layer - 1) * H:layer * H],
                                start=True, stop=True)
                            zt = zp.tile([128, 128], F16, tag="zt")
                            nc.scalar.copy(zt[:, :H], pz[:])
                            nc.sync.dma_start(
                                bounces[layer][t * 128:(t + 1) * 128, :], zt[:])
                        else:
                            p5 = ps_agg.tile([128, H], F32, space="PSUM", tag="pT")
                            for j, col in enumerate(cols):
                                nc.tensor.matmul(
                                    out=p5[:], lhsT=m01[:, col, :],
                                    rhs=g[:, col, 0:H],
                                    start=(j == 0), stop=(j == len(cols) - 1))
                            h5 = hp.tile([128, H], F16, tag="hT")
                            nc.vector.tensor_tensor(out=h5[:], in0=p5[:],
                                                    in1=b5rep_sb[:],
                                                    op=mybir.AluOpType.add)
                            nc.scalar.activation(h5[:], h5[:],
                                                 mybir.ActivationFunctionType.Relu)
                            pt = pop.tile([128, 128], F16, tag="pt")
                            nc.vector.tensor_tensor(
                                out=pt[:], in0=iota16_sb[:],
                                in1=gcol_sb[:, t:t + 1].to_broadcast([128, 128]),
                                op=mybir.AluOpType.is_equal)
                            nc.tensor.matmul(
                                out=pool_ps[:], lhsT=h5[:], rhs=pt[:],
                                start=(pool_i == 0), stop=(pool_i == n_pool - 1))
                            pool_i += 1
                if layer < 5:
                    nc.gpsimd.collective_compute(
                        "AllGather", mybir.AluOpType.bypass,
                        replica_groups=[list(range(M))],
                        ins=[bounces[layer][:]], outs=[zfulls[layer][:]])

            # transpose, scale by 1/cnt, layernorm, output head (f32 tail)
            pool_sb = ep.tile([H, 128], F32, tag="poolsb")
            nc.scalar.copy(pool_sb[:], pool_ps[:])
            ptr = ps_z.tile([128, H], F32, space="PSUM", tag="pz")
            nc.tensor.transpose(out=ptr[:], in_=pool_sb[:], identity=ident_sb[:])
            pooled = ep.tile([128, H], F32, tag="pooled")
            nc.vector.tensor_scalar(out=pooled[:], in0=ptr[:],
                                    scalar1=invcnt_sb[:, 0:1], scalar2=None,
                                    op0=mybir.AluOpType.mult)
            mu = ep.tile([128, 1], F32, tag="mu")
            nc.vector.tensor_reduce(out=mu[:], in_=pooled[:],
                                    axis=mybir.AxisListType.X,
                                    op=mybir.AluOpType.add)
            nc.vector.tensor_scalar(out=mu[:], in0=mu[:], scalar1=1.0 / H,
                                    scalar2=None, op0=mybir.AluOpType.mult)
            xc = ep.tile([128, H], F32, tag="xc")
            nc.vector.tensor_scalar(out=xc[:], in0=pooled[:], scalar1=mu[:, 0:1],
                                    scalar2=None, op0=mybir.AluOpType.subtract)
            sq = ep.tile([128, H], F32, tag="sq")
            nc.scalar.activation(sq[:], xc[:], mybir.ActivationFunctionType.Square)
            var = ep.tile([128, 1], F32, tag="var")
            nc.vector.tensor_reduce(out=var[:], in_=sq[:],
                                    axis=mybir.AxisListType.X,
                                    op=mybir.AluOpType.add)
            nc.vector.tensor_scalar(out=var[:], in0=var[:], scalar1=1.0 / H,
                                    scalar2=None, op0=mybir.AluOpType.mult)
            eps_col = ep.tile([128, 1], F32, tag="eps")
            nc.vector.memset(eps_col[:], 1e-5)
            std = ep.tile([128, 1], F32, tag="std")
            nc.scalar.activation(std[:], var[:], mybir.ActivationFunctionType.Sqrt,
                                 bias=eps_col[:, 0:1])
            rstd = ep.tile([128, 1], F32, tag="rstd")
            nc.vector.reciprocal(rstd[:], std[:])
            ln = ep.tile([128, H], F32, tag="ln")
            nc.vector.tensor_scalar(out=ln[:], in0=xc[:], scalar1=rstd[:, 0:1],
                                    scalar2=None, op0=mybir.AluOpType.mult)
            y = ep.tile([128, H], F32, tag="y")
            nc.vector.tensor_tensor(out=y[:], in0=ln[:], in1=woutrep_sb[:],
                                    op=mybir.AluOpType.mult)
            yr = ep.tile([128, 1], F32, tag="yr")
            nc.vector.tensor_reduce(out=yr[:], in_=y[:], axis=mybir.AxisListType.X,
                                    op=mybir.AluOpType.add)
            nc.vector.tensor_scalar(out=yr[:], in0=yr[:], scalar1=bout,
                                    scalar2=None, op0=mybir.AluOpType.add)
            nc.sync.dma_start(out_d[:], yr[:])

    from concourse.library_overlay import lower_extended_insts
    lower_extended_insts(nc)
    return nc


# ---------------------------------------------------------------------------
# Entry point
# ---------------------------------------------------------------------------

def kernel(x, edge_index, batch, W1, b1, Wh, bh, Wout, bout):
    from concourse.bass_utils import run_bass_kernel_spmd

    x = np.asarray(x, np.float32)
    edge_index = np.asarray(edge_index)
    batch = np.asarray(batch)
    n_graphs = 1000

    in_maps, meta = preprocess(x, edge_index, batch, n_graphs)
    wmaps, wmeta = make_weight_inputs(W1, b1, Wh, bh, Wout, bout)
    nc = build_nc(meta, dict(bout=wmeta["bout"]))
    for im in in_maps:
        im.update(wmaps)

    import time
    last_err = None
    for attempt in range(3):
        try:
            res = run_bass_kernel_spmd(nc, in_maps, core_ids=list(range(M)))
            break
        except Exception as e:  # transient terminal hiccups / device recovery
            last_err = e
            time.sleep(30 * (attempt + 1))
    else:
        raise last_err

    GPC = meta["GPC"]
    out = np.concatenate([res.results[c]["out"][:GPC] for c in range(M)], axis=0)
    return np.ascontiguousarray(out, np.float32)


# revision 27
# speedup vs baseline: 1.4835x; 1.1370x over previous
"""GCN-5 message-passing kernel for Trainium2, 8-core SPMD Bass/Tile.

Strategy (graph-parallel per the sharding hint), v2:
  - batch is sorted, so graphs are contiguous node ranges.  Core c owns graphs
    [125c, 125(c+1)) and their nodes; edges are assigned to the core owning the
    dst node.  Pooling / layernorm / output head are fully local per core.
  - Everything on the edge path is fp16 (tolerance is 2e-2): z rows are stored
    padded to 128 fp16 (256B) so one gather descriptor moves a full row.
  - Per layer: z = h @ W per local tile on the TensorEngine (fp16, 1 cyc/row
    vs fp32's 4), AllGather (fp16, same bytes as the old f32x64), then batched
    `dma_gather` (InstDMAGatherAnt, ~1us fixed + 0.34ns/row) replaces the old
    per-128-edge indirect_dma_start (~1.2us each) -- descriptor generation on
    GPSIMD drops from ~7ms to ~0.6ms.
  - dma_gather indices are int16, so the 101376-row z table is bucketed into
    4 ranges of 2 cores (2*S <= 32767 rows); edges are grouped per
    (tile-chunk, bucket) with uniform-across-cores padded block counts (the
    SPMD program is shared, so counts are max'd over cores; pads carry idx=0
    and norm=0).  Calls are capped at 4096 idxs with single_packet=False
    (larger/packed calls wedge the SWDGE ring on this runtime).
  - Scatter-add per 128-dst tile stays a one-hot matmul (fp16) with PSUM
    accumulation; the one-hot is built per chunk in one DVE is_equal.
  - Layer 5 aggregates in [node, H] orientation and feeds the per-graph
    mean-pool one-hot matmul directly, accumulating all tiles in one PSUM
    bank; layernorm + head are unchanged f32.
"""
import sys
import types
import contextlib

import numpy as np

sys.path.insert(0, "/opt/trn_rl_repo")

import concourse.bass as bass
import concourse.tile as tile
from concourse import mybir, library_config
from concourse.vector_clock import ScopedClock

F32 = mybir.dt.float32
F16 = mybir.dt.float16
I16 = mybir.dt.int16
M = 8  # NeuronCores
H = 64
CH = 8          # tiles per gather chunk
CALL_MAX = 4096  # max idxs per dma_gather call

# ---------------------------------------------------------------------------
# Environment fixes for this container
# ---------------------------------------------------------------------------

def _install_env_fixes():
    import concourse.tile as tile_mod

    def _patched_drain_and_barrier(self, tick_clock, wait_clock):
        # this walrus build allows a single sync-wait per TPB_CTRL Drain;
        # split the Tile tail-drain's waits across multiple drains.
        nc = self.nc
        drain_inst = nc.sync.drain()
        wait_clock.add_sem_waits(drain_inst.ins,
                                 ScopedClock({None: tick_clock.global_clock}))
        si = drain_inst.ins.sync_info
        waits = list(si.on_wait or [])
        if len(waits) > 1:
            si.on_wait[:] = waits[:1]
            for w in waits[1:]:
                d2 = nc.sync.drain()
                if d2.ins.sync_info is None:
                    d2.ins.sync_info = mybir.SyncInfo(on_wait=[w], on_update=[])
                else:
                    d2.ins.sync_info.on_wait.append(w)
        nc.all_engine_barrier()
        assert self.sems is not None
        popped = nc._tile_sem_poison_stack.pop()
        assert popped is self._sem_poison
        nc.clear_and_free_semaphores(list(self.sems.allocated().values()))
        nc.all_engine_barrier()

    tile_mod.TileContext._drain_and_barrier = _patched_drain_and_barrier

    _orig_lower_ordered = tile_mod.TileContext._lower_ordered_insts

    def _split_multiwait_lower(self, ordered):
        nc = self.nc
        for bbname, insts in ordered.items():
            newlist = []
            changed = False
            for inst in insts:
                si = getattr(inst, "sync_info", None)
                eng = getattr(inst, "engine", None)
                if (si is not None and si.on_wait and len(si.on_wait) > 1
                        and eng is not None and eng != mybir.EngineType.Unassigned
                        and inst.is_executable()):
                    waits = list(si.on_wait)
                    si.on_wait[:] = waits[-1:]
                    for w in waits[:-1]:
                        nop = mybir.InstNoOp(
                            name=nc.get_next_instruction_name(), engine=eng)
                        nop.sync_info = mybir.SyncInfo(on_wait=[w], on_update=[])
                        try:
                            nc.register_instruction(nop, overwrite=True)
                        except Exception:
                            pass
                        newlist.append(nop)
                    changed = True
                newlist.append(inst)
            if changed:
                insts[:] = newlist
        return _orig_lower_ordered(self, ordered)

    if getattr(tile_mod.TileContext._lower_ordered_insts, "__name__", "") != \
            "_split_multiwait_lower":
        tile_mod.TileContext._lower_ordered_insts = _split_multiwait_lower

    # NTFF profile hook (lets trace=True work under axon); best-effort.
    if "antenv.axon_hooks" not in sys.modules:
        try:
            from trn_agent_boot.trn_boot import _ntff_profile_via_ctypes
            hook = _ntff_profile_via_ctypes("/opt/axon/libaxon_pjrt.so")
            mod = types.ModuleType("antenv.axon_hooks")
            mod.get_axon_ntff_profile_hook = lambda: hook
            mod.set_axon_ntff_profile_hook = lambda h: None
            sys.modules["antenv.axon_hooks"] = mod
            import antenv
            antenv.axon_hooks = mod
        except Exception:
            pass


_install_env_fixes()

# ---------------------------------------------------------------------------
# Host preprocessing
# ---------------------------------------------------------------------------

def preprocess(x, edge_index, batch, n_graphs):
    N = x.shape[0]
    GPC = n_graphs // M
    e0 = np.asarray(edge_index[0], np.int64)
    e1 = np.asarray(edge_index[1], np.int64)
    # self-loops are excluded from the gathered edge stream (handled as a
    # cheap identity-matmul term from the SBUF-resident local z); they still
    # count toward the degrees
    src = e0
    dst = e1
    deg = (np.bincount(e1, minlength=N) + 1).astype(np.float32)
    dis = 1.0 / np.sqrt(deg)
    norm = (dis[src] * dis[dst]).astype(np.float32)
    selfnorm = (dis * dis).astype(np.float32)

    batch = np.asarray(batch, np.int64)
    owner = batch // GPC
    node_start = np.searchsorted(batch, np.arange(M) * GPC)
    node_end = np.searchsorted(batch, np.arange(M) * GPC + GPC)
    n_c = node_end - node_start
    T = int(np.ceil(n_c.max() / 128))
    S = T * 128
    assert 2 * S <= 32767, (T, S)
    local = np.arange(N) - node_start[owner]
    row = owner * S + local

    eo = owner[dst]
    ld = local[dst]
    et = ld // 128
    ep = (ld % 128).astype(np.float32)
    er = row[src]
    eb = er // (2 * S)
    erel = er - eb * (2 * S)

    # uniform (max-over-cores) padded block counts per (tile, bucket)
    key = ((eo * T + et) * 4 + eb)
    counts = np.bincount(key, minlength=M * T * 4).reshape(M, T, 4)
    nblk_tb = -(-counts.max(axis=0) // 128)  # [T, 4]

    nchunks = -(-T // CH)
    blk_of_tb = np.zeros((T, 4), np.int64)
    chunk_info = []
    call_tb = []  # (t, b) per call, in program order, for the count tensor
    gb = 0
    for k in range(nchunks):
        t0, t1 = k * CH, min((k + 1) * CH, T)
        blk0 = gb
        calls = []
        for b in range(4):
            rblk0 = gb
            for t in range(t0, t1):
                blk_of_tb[t, b] = gb
                gb += int(nblk_tb[t, b])
            nreg = gb - rblk0
            off = 0
            while off < nreg:
                nb = min(nreg - off, CALL_MAX // 128)
                calls.append((b, (rblk0 + off) * 128, nb * 128,
                              (rblk0 + off) - blk0, len(call_tb)))
                call_tb.append(None)
                off += nb
        tiles = []
        for t in range(t0, t1):
            cols = []
            for b in range(4):
                st = int(blk_of_tb[t, b] - blk0)
                cols.extend(range(st, st + int(nblk_tb[t, b])))
            tiles.append((t, cols))
        chunk_info.append(dict(blk0=blk0, nblk=gb - blk0, calls=calls,
                               tiles=tiles))
    total_blocks = gb
    total_slots = gb * 128
    ncalls = len(call_tb)

    # per-core slot arrays
    ordk = np.argsort(key, kind="stable")
    ks = key[ordk]
    group_start = np.zeros(M * T * 4 + 1, np.int64)
    np.cumsum(counts.reshape(-1), out=group_start[1:])
    pos = np.arange(len(ks)) - group_start[ks]
    slot_base = blk_of_tb.reshape(-1)
    slot = slot_base[ks % (T * 4)] * 128 + pos
    core_of = ks // (T * 4)

    er_s = erel[ordk]
    nm_s = norm[ordk]
    ep_s = ep[ordk]

    cnt_g = np.bincount(batch, minlength=n_graphs).astype(np.float32)

    in_maps = []
    for c in range(M):
        selc = core_of == c
        sl = slot[selc]
        idx16 = np.zeros(total_slots, np.int16)
        nrm = np.zeros(total_slots, np.float16)
        lds = np.zeros(total_slots, np.float16)
        idx16[sl] = er_s[selc].astype(np.int16)
        nrm[sl] = nm_s[selc].astype(np.float16)
        lds[sl] = ep_s[selc].astype(np.float16)
        idxw = np.ascontiguousarray(
            np.tile(idx16.reshape(-1, 16).T, (8, 1)))
        nrm2 = np.ascontiguousarray(nrm.reshape(-1, 128).T)
        lds2 = np.ascontiguousarray(lds.reshape(-1, 128).T)

        xT = np.zeros((128, S), np.float16)
        xT[:, :n_c[c]] = x[node_start[c]:node_end[c]].T

        gflat = np.full(T * 128, -1.0, np.float16)
        gflat[:n_c[c]] = (batch[node_start[c]:node_end[c]] - c * GPC)
        gcol = np.ascontiguousarray(gflat.reshape(T, 128).T)

        sn = np.zeros(T * 128, np.float32)
        sn[:n_c[c]] = selfnorm[node_start[c]:node_end[c]]
        snorm = np.ascontiguousarray(sn.reshape(T, 128).T)

        invcnt = np.zeros((128, 1), np.float32)
        invcnt[:GPC, 0] = 1.0 / np.maximum(cnt_g[c * GPC:(c + 1) * GPC], 1.0)

        in_maps.append(dict(idx=idxw, nrm=nrm2, lds=lds2, xT=xT, gcol=gcol,
                            invcnt=invcnt, snorm=snorm))
    meta = dict(T=T, S=S, GPC=GPC, chunks=chunk_info,
                total_blocks=total_blocks, total_slots=total_slots,
                ncalls=ncalls)
    return in_maps, meta


def make_weight_inputs(W1, b1, Wh, bh, Wout, bout):
    Wh = np.asarray(Wh, np.float32)
    iota16 = np.broadcast_to(np.arange(128, dtype=np.float16), (128, 128)).copy()
    ident = np.eye(H, dtype=np.float32)
    ident128 = np.eye(128, dtype=np.float16)
    return dict(
        ident128=ident128,
        W1=np.ascontiguousarray(W1, np.float32).astype(np.float16),
        Wh2=np.ascontiguousarray(
            Wh.transpose(1, 0, 2).reshape(H, 4 * H)).astype(np.float16),
        b14=np.ascontiguousarray(
            np.stack([b1, bh[0], bh[1], bh[2]], axis=1), np.float32),
        b5rep=np.broadcast_to(np.asarray(bh[3], np.float32), (128, H)).copy(),
        woutrep=np.broadcast_to(np.asarray(Wout, np.float32)[:, 0], (128, H)).copy(),
        iota16=iota16,
        ident=ident,
    ), dict(bout=np.asarray(bout, np.float32))


# ---------------------------------------------------------------------------
# Bass program
# ---------------------------------------------------------------------------

def build_nc(meta, weights):
    T, S = meta["T"], meta["S"]
    chunks = meta["chunks"]
    total_blocks = meta["total_blocks"]
    total_slots = meta["total_slots"]
    ncalls = meta["ncalls"]
    nc = bass.Bass("TRN2", target_bir_lowering=False)

    xT_d = nc.declare_dram_parameter("xT", [128, S], F16, isOutput=False)
    idx_d = nc.declare_dram_parameter("idx", [128, total_slots // 16], I16,
                                      isOutput=False)
    nrm_d = nc.declare_dram_parameter("nrm", [128, total_blocks], F16,
                                      isOutput=False)
    lds_d = nc.declare_dram_parameter("lds", [128, total_blocks], F16,
                                      isOutput=False)
    gcol_d = nc.declare_dram_parameter("gcol", [128, T], F16, isOutput=False)
    invcnt_d = nc.declare_dram_parameter("invcnt", [128, 1], F32, isOutput=False)
    iota16_d = nc.declare_dram_parameter("iota16", [128, 128], F16, isOutput=False)
    W1_d = nc.declare_dram_parameter("W1", [128, H], F16, isOutput=False)
    Wh2_d = nc.declare_dram_parameter("Wh2", [H, 4 * H], F16, isOutput=False)
    b14_d = nc.declare_dram_parameter("b14", [H, 4], F32, isOutput=False)
    b5rep_d = nc.declare_dram_parameter("b5rep", [128, H], F32, isOutput=False)
    woutrep_d = nc.declare_dram_parameter("woutrep", [128, H], F32, isOutput=False)
    ident_d = nc.declare_dram_parameter("ident", [H, H], F32, isOutput=False)
    ident128_d = nc.declare_dram_parameter("ident128", [128, 128], F16,
                                           isOutput=False)
    snorm_d = nc.declare_dram_parameter("snorm", [128, T], F32, isOutput=False)
    out_d = nc.declare_dram_parameter("out", [128, 1], F32, isOutput=True)
    bout = float(weights["bout"][0])

    with tile.TileContext(nc) as tc:
        with contextlib.ExitStack() as ctx:
            zfull_pool = ctx.enter_context(
                tc.tile_pool(name="zfull", bufs=1, space="DRAM"))
            dram = ctx.enter_context(tc.tile_pool(name="dram", bufs=1, space="DRAM"))
            const = ctx.enter_context(tc.tile_pool(name="const", bufs=1))
            xp = ctx.enter_context(tc.tile_pool(name="xp", bufs=3))
            gp = ctx.enter_context(tc.tile_pool(name="gp", bufs=2))
            mp = ctx.enter_context(tc.tile_pool(name="mp", bufs=2))
            hp = ctx.enter_context(tc.tile_pool(name="hp", bufs=3))
            zp = ctx.enter_context(tc.tile_pool(name="zp", bufs=3))
            pop = ctx.enter_context(tc.tile_pool(name="pop", bufs=3))
            ep = ctx.enter_context(tc.tile_pool(name="ep", bufs=1))
            ps_agg = ctx.enter_context(tc.tile_pool(name="ps_agg", bufs=2, space="PSUM"))
            ps_z = ctx.enter_context(tc.tile_pool(name="ps_z", bufs=2, space="PSUM"))
            ps_pool = ctx.enter_context(tc.tile_pool(name="ps_pool", bufs=1, space="PSUM"))

            nc.gpsimd.load_library(library_config.mlp)
            # one shared register for dma_gather's num_idxs (to_reg would
            # allocate a fresh register per call and exhaust the file)
            nidx_reg = nc.alloc_register(mybir.EngineType.Pool, "nidx_reg")

            zfulls = [zfull_pool.tile([M * S, 128], F16, addr_space="Shared",
                                      name=f"zfull{k}", tag=f"zfull{k}")
                      for k in range(5)]
            bounces = [dram.tile([S, 128], F16, name=f"bounce{k}", tag=f"bounce{k}")
                       for k in range(5)]

            idx_sb = const.tile([128, total_slots // 16], I16)
            nc.sync.dma_start(idx_sb[:], idx_d[:])
            nrm_sb = const.tile([128, total_blocks], F16)
            nc.sync.dma_start(nrm_sb[:], nrm_d[:])
            lds_sb = const.tile([128, total_blocks], F16)
            nc.sync.dma_start(lds_sb[:], lds_d[:])
            gcol_sb = const.tile([128, T], F16)
            nc.sync.dma_start(gcol_sb[:], gcol_d[:])
            invcnt_sb = const.tile([128, 1], F32)
            nc.sync.dma_start(invcnt_sb[:], invcnt_d[:])
            iota16_sb = const.tile([128, 128], F16)
            nc.sync.dma_start(iota16_sb[:], iota16_d[:])
            W1_sb = const.tile([128, H], F16)
            nc.sync.dma_start(W1_sb[:], W1_d[:])
            Wh2_sb = const.tile([H, 4 * H], F16)
            nc.sync.dma_start(Wh2_sb[:], Wh2_d[:])
            b14_sb = const.tile([H, 4], F32)
            nc.sync.dma_start(b14_sb[:], b14_d[:])
            b5rep_sb = const.tile([128, H], F32)
            nc.sync.dma_start(b5rep_sb[:], b5rep_d[:])
            woutrep_sb = const.tile([128, H], F32)
            nc.sync.dma_start(woutrep_sb[:], woutrep_d[:])
            ident_sb = const.tile([H, H], F32)
            nc.sync.dma_start(ident_sb[:], ident_d[:])
            ident128_sb = const.tile([128, 128], F16)
            nc.sync.dma_start(ident128_sb[:], ident128_d[:])
            snorm_sb = const.tile([128, T], F32)
            nc.sync.dma_start(snorm_sb[:], snorm_d[:])
            # the core's own z tiles for the current layer (self-loop source)
            zall = const.tile([128, T * H], F16)

            # layer 0: z0 = x @ W1 per local tile, then AllGather
            for t in range(T):
                xt = xp.tile([128, 128], F16, tag="xt")
                nc.sync.dma_start(xt[:], xT_d[:, t * 128:(t + 1) * 128])
                pz = ps_z.tile([128, H], F32, space="PSUM", tag="pz")
                nc.tensor.matmul(out=pz[:], lhsT=xt[:], rhs=W1_sb[:],
                                 start=True, stop=True)
                zt = zp.tile([128, 128], F16, tag="zt")
                nc.scalar.copy(zt[:, :H], pz[:])
                nc.scalar.copy(zall[:, t * H:(t + 1) * H], pz[:])
                nc.sync.dma_start(bounces[0][t * 128:(t + 1) * 128, :], zt[:])
            nc.gpsimd.collective_compute(
                "AllGather", mybir.AluOpType.bypass,
                replica_groups=[list(range(M))],
                ins=[bounces[0][:]], outs=[zfulls[0][:]])

            n_pool = sum(1 for ci in chunks for (t, cols) in ci["tiles"])
            pool_ps = None
            pool_i = 0

            for layer in range(1, 6):
                zf = zfulls[layer - 1]
                if layer == 5:
                    pool_ps = ps_pool.tile([H, 128], F32, space="PSUM", tag="pool")
                for ci in chunks:
                    cb = ci["nblk"]
                    blk0 = ci["blk0"]
                    g = gp.tile([128, cb, 128], F16, tag="g")
                    for (b, slot0, nidx, blkoff, call_i) in ci["calls"]:
                        nc.gpsimd.reg_mov(nidx_reg, nidx)
                        nc.gpsimd.dma_gather(
                            g[:, blkoff:blkoff + nidx // 128, :],
                            zf[b * 2 * S:(b + 1) * 2 * S, :],
                            idx_sb[:, slot0 // 16:(slot0 + nidx) // 16],
                            nidx, nidx_reg, 128, single_packet=False)
                    gs = g[:, :, 0:H]
                    nrm3 = nrm_sb[:, blk0:blk0 + cb, None].to_broadcast([128, cb, H])
                    nc.vector.tensor_tensor(out=gs, in0=gs, in1=nrm3,
                                            op=mybir.AluOpType.mult)
                    m01 = mp.tile([128, cb, 128], F16, tag="m01")
                    iota3 = iota16_sb[:, None, :].to_broadcast([128, cb, 128])
                    lds3 = lds_sb[:, blk0:blk0 + cb, None].to_broadcast([128, cb, 128])
                    nc.vector.tensor_tensor(out=m01[:], in0=iota3, in1=lds3,
                                            op=mybir.AluOpType.is_equal)
                    for (t, cols) in ci["tiles"]:
                        selfmsg = hp.tile([128, H], F16, tag="selfmsg")
                        nc.vector.tensor_scalar(
                            out=selfmsg[:], in0=zall[:, t * H:(t + 1) * H],
                            scalar1=snorm_sb[:, t:t + 1], scalar2=None,
                            op0=mybir.AluOpType.mult)
                        if layer < 5:
                            pT = ps_agg.tile([H, 128], F32, space="PSUM", tag="pT")
                            for j, col in enumerate(cols):
                                nc.tensor.matmul(
                                    out=pT[:], lhsT=g[:, col, 0:H],
                                    rhs=m01[:, col, :],
                                    start=(j == 0), stop=False)
                            nc.tensor.matmul(
                                out=pT[:], lhsT=selfmsg[:], rhs=ident128_sb[:],
                                start=(len(cols) == 0), stop=True)
                            hT = hp.tile([H, 128], F16, tag="hT")
                            nc.scalar.activation(
                                hT[:], pT[:], mybir.ActivationFunctionType.Relu,
                                bias=b14_sb[:, layer - 1:layer])
                            pz = ps_z.tile([128, H], F32, space="PSUM", tag="pz")
                            nc.tensor.matmul(
                                out=pz[:], lhsT=hT[:],
                                rhs=Wh2_sb[:, (layer - 1) * H:layer * H],
                                start=True, stop=True)
                            zt = zp.tile([128, 128], F16, tag="zt")
                            nc.scalar.copy(zt[:, :H], pz[:])
                            nc.scalar.copy(zall[:, t * H:(t + 1) * H], pz[:])
                            nc.sync.dma_start(
                                bounces[layer][t * 128:(t + 1) * 128, :], zt[:])
                        else:
                            p5 = ps_agg.tile([128, H], F32, space="PSUM", tag="pT")
                            for j, col in enumerate(cols):
                                nc.tensor.matmul(
                                    out=p5[:], lhsT=m01[:, col, :],
                                    rhs=g[:, col, 0:H],
                                    start=(j == 0), stop=False)
                            nc.tensor.matmul(
                                out=p5[:], lhsT=ident128_sb[:], rhs=selfmsg[:],
                                start=(len(cols) == 0), stop=True)
                            h5 = hp.tile([128, H], F16, tag="hT")
                            nc.vector.tensor_tensor(out=h5[:], in0=p5[:],
                                                    in1=b5rep_sb[:],
                                                    op=mybir.AluOpType.add)
                            nc.scalar.activation(h5[:], h5[:],
                                                 mybir.ActivationFunctionType.Relu)
                            pt = pop.tile([128, 128], F16, tag="pt")
                            nc.vector.tensor_tensor(
                                out=pt[:], in0=iota16_sb[:],
                                in1=gcol_sb[:, t:t + 1].to_broadcast([128, 128]),
                                op=mybir.AluOpType.is_equal)
                            nc.tensor.matmul(
                                out=pool_ps[:], lhsT=h5[:], rhs=pt[:],
                                start=(pool_i == 0), stop=(pool_i == n_pool - 1))
                            pool_i += 1
                if layer < 5:
                    nc.gpsimd.collective_compute(
                        "AllGather", mybir.AluOpType.bypass,
                        replica_groups=[list(range(M))],
                        ins=[bounces[layer][:]], outs=[zfulls[layer][:]])

            # transpose, scale by 1/cnt, layernorm, output head (f32 tail)
            pool_sb = ep.tile([H, 128], F32, tag="poolsb")
            nc.scalar.copy(pool_sb[:], pool_ps[:])
            ptr = ps_z.tile([128, H], F32, space="PSUM", tag="pz")
            nc.tensor.transpose(out=ptr[:], in_=pool_sb[:], identity=ident_sb[:])
            pooled = ep.tile([128, H], F32, tag="pooled")
            nc.vector.tensor_scalar(out=pooled[:], in0=ptr[:],
                                    scalar1=invcnt_sb[:, 0:1], scalar2=None,
                                    op0=mybir.AluOpType.mult)
            mu = ep.tile([128, 1], F32, tag="mu")
            nc.vector.tensor_reduce(out=mu[:], in_=pooled[:],
                                    axis=mybir.AxisListType.X,
                                    op=mybir.AluOpType.add)
            nc.vector.tensor_scalar(out=mu[:], in0=mu[:], scalar1=1.0 / H,
                                    scalar2=None, op0=mybir.AluOpType.mult)
            xc = ep.tile([128, H], F32, tag="xc")
            nc.vector.tensor_scalar(out=xc[:], in0=pooled[:], scalar1=mu[:, 0:1],
                                    scalar2=None, op0=mybir.AluOpType.subtract)
            sq = ep.tile([128, H], F32, tag="sq")
            nc.scalar.activation(sq[:], xc[:], mybir.ActivationFunctionType.Square)
            var = ep.tile([128, 1], F32, tag="var")
            nc.vector.tensor_reduce(out=var[:], in_=sq[:],
                                    axis=mybir.AxisListType.X,
                                    op=mybir.AluOpType.add)
            nc.vector.tensor_scalar(out=var[:], in0=var[:], scalar1=1.0 / H,
                                    scalar2=None, op0=mybir.AluOpType.mult)
            eps_col = ep.tile([128, 1], F32, tag="eps")
            nc.vector.memset(eps_col[:], 1e-5)
            std = ep.tile([128, 1], F32, tag="std")
            nc.scalar.activation(std[:], var[:], mybir.ActivationFunctionType.Sqrt,
                                 bias=eps_col[:, 0:1])
            rstd = ep.tile([128, 1], F32, tag="rstd")
            nc.vector.reciprocal(rstd[:], std[:])
            ln = ep.tile([128, H], F32, tag="ln")
            nc.vector.tensor_scalar(out=ln[:], in0=xc[:], scalar1=rstd[:, 0:1],
                                    scalar2=None, op0=mybir.AluOpType.mult)
            y = ep.tile([128, H], F32, tag="y")
            nc.vector.tensor_tensor(out=y[:], in0=ln[:], in1=woutrep_sb[:],
                                    op=mybir.AluOpType.mult)
            yr = ep.tile([128, 1], F32, tag="yr")
            nc.vector.tensor_reduce(out=yr[:], in_=y[:], axis=mybir.AxisListType.X,
                                    op=mybir.AluOpType.add)
            nc.vector.tensor_scalar(out=yr[:], in0=yr[:], scalar1=bout,
                                    scalar2=None, op0=mybir.AluOpType.add)
            nc.sync.dma_start(out_d[:], yr[:])

    from concourse.library_overlay import lower_extended_insts
    lower_extended_insts(nc)
    return nc


# ---------------------------------------------------------------------------
# Entry point
# ---------------------------------------------------------------------------

def kernel(x, edge_index, batch, W1, b1, Wh, bh, Wout, bout):
    from concourse.bass_utils import run_bass_kernel_spmd

    x = np.asarray(x, np.float32)
    edge_index = np.asarray(edge_index)
    batch = np.asarray(batch)
    n_graphs = 1000

    in_maps, meta = preprocess(x, edge_index, batch, n_graphs)
    wmaps, wmeta = make_weight_inputs(W1, b1, Wh, bh, Wout, bout)
    nc = build_nc(meta, dict(bout=wmeta["bout"]))
    for im in in_maps:
        im.update(wmaps)

    import time
    last_err = None
    for attempt in range(3):
        try:
            res = run_bass_kernel_spmd(nc, in_maps, core_ids=list(range(M)))
            break
        except Exception as e:  # transient terminal hiccups / device recovery
            last_err = e
            time.sleep(30 * (attempt + 1))
    else:
        raise last_err

    GPC = meta["GPC"]
    out = np.concatenate([res.results[c]["out"][:GPC] for c in range(M)], axis=0)
    return np.ascontiguousarray(out, np.float32)


# revision 32
# speedup vs baseline: 1.5923x; 1.0733x over previous
"""GCN-5 message-passing kernel for Trainium2, 8-core SPMD Bass/Tile.

Strategy (graph-parallel per the sharding hint), v2:
  - batch is sorted, so graphs are contiguous node ranges.  Core c owns graphs
    [125c, 125(c+1)) and their nodes; edges are assigned to the core owning the
    dst node.  Pooling / layernorm / output head are fully local per core.
  - Everything on the edge path is fp16 (tolerance is 2e-2): z rows are stored
    padded to 128 fp16 (256B) so one gather descriptor moves a full row.
  - Per layer: z = h @ W per local tile on the TensorEngine (fp16, 1 cyc/row
    vs fp32's 4), AllGather (fp16, same bytes as the old f32x64), then batched
    `dma_gather` (InstDMAGatherAnt, ~1us fixed + 0.34ns/row) replaces the old
    per-128-edge indirect_dma_start (~1.2us each) -- descriptor generation on
    GPSIMD drops from ~7ms to ~0.6ms.
  - dma_gather indices are int16, so the 101376-row z table is bucketed into
    4 ranges of 2 cores (2*S <= 32767 rows); edges are grouped per
    (tile-chunk, bucket) with uniform-across-cores padded block counts (the
    SPMD program is shared, so counts are max'd over cores; pads carry idx=0
    and norm=0).  Calls are capped at 4096 idxs with single_packet=False
    (larger/packed calls wedge the SWDGE ring on this runtime).
  - Scatter-add per 128-dst tile stays a one-hot matmul (fp16) with PSUM
    accumulation; the one-hot is built per chunk in one DVE is_equal.
  - Layer 5 aggregates in [node, H] orientation and feeds the per-graph
    mean-pool one-hot matmul directly, accumulating all tiles in one PSUM
    bank; layernorm + head are unchanged f32.
"""
import sys
import types
import contextlib

import numpy as np

sys.path.insert(0, "/opt/trn_rl_repo")

import concourse.bass as bass
import concourse.tile as tile
from concourse import mybir, library_config
from concourse.vector_clock import ScopedClock

F32 = mybir.dt.float32
F16 = mybir.dt.float16
I16 = mybir.dt.int16
M = 8  # NeuronCores
H = 64
CH = 8          # tiles per gather chunk
CALL_MAX = 4096  # max idxs per dma_gather call

# ---------------------------------------------------------------------------
# Environment fixes for this container
# ---------------------------------------------------------------------------

def _install_env_fixes():
    import concourse.tile as tile_mod

    def _patched_drain_and_barrier(self, tick_clock, wait_clock):
        # this walrus build allows a single sync-wait per TPB_CTRL Drain;
        # split the Tile tail-drain's waits across multiple drains.
        nc = self.nc
        drain_inst = nc.sync.drain()
        wait_clock.add_sem_waits(drain_inst.ins,
                                 ScopedClock({None: tick_clock.global_clock}))
        si = drain_inst.ins.sync_info
        waits = list(si.on_wait or [])
        if len(waits) > 1:
            si.on_wait[:] = waits[:1]
            for w in waits[1:]:
                d2 = nc.sync.drain()
                if d2.ins.sync_info is None:
                    d2.ins.sync_info = mybir.SyncInfo(on_wait=[w], on_update=[])
                else:
                    d2.ins.sync_info.on_wait.append(w)
        nc.all_engine_barrier()
        assert self.sems is not None
        popped = nc._tile_sem_poison_stack.pop()
        assert popped is self._sem_poison
        nc.clear_and_free_semaphores(list(self.sems.allocated().values()))
        nc.all_engine_barrier()

    tile_mod.TileContext._drain_and_barrier = _patched_drain_and_barrier

    _orig_lower_ordered = tile_mod.TileContext._lower_ordered_insts

    def _split_multiwait_lower(self, ordered):
        nc = self.nc
        for bbname, insts in ordered.items():
            newlist = []
            changed = False
            for inst in insts:
                si = getattr(inst, "sync_info", None)
                eng = getattr(inst, "engine", None)
                if (si is not None and si.on_wait and len(si.on_wait) > 1
                        and eng is not None and eng != mybir.EngineType.Unassigned
                        and inst.is_executable()):
                    waits = list(si.on_wait)
                    si.on_wait[:] = waits[-1:]
                    for w in waits[:-1]:
                        nop = mybir.InstNoOp(
                            name=nc.get_next_instruction_name(), engine=eng)
                        nop.sync_info = mybir.SyncInfo(on_wait=[w], on_update=[])
                        try:
                            nc.register_instruction(nop, overwrite=True)
                        except Exception:
                            pass
                        newlist.append(nop)
                    changed = True
                newlist.append(inst)
            if changed:
                insts[:] = newlist
        return _orig_lower_ordered(self, ordered)

    if getattr(tile_mod.TileContext._lower_ordered_insts, "__name__", "") != \
            "_split_multiwait_lower":
        tile_mod.TileContext._lower_ordered_insts = _split_multiwait_lower

    # NTFF profile hook (lets trace=True work under axon); best-effort.
    if "antenv.axon_hooks" not in sys.modules:
        try:
            from trn_agent_boot.trn_boot import _ntff_profile_via_ctypes
            hook = _ntff_profile_via_ctypes("/opt/axon/libaxon_pjrt.so")
            mod = types.ModuleType("antenv.axon_hooks")
            mod.get_axon_ntff_profile_hook = lambda: hook
            mod.set_axon_ntff_profile_hook = lambda h: None
            sys.modules["antenv.axon_hooks"] = mod
            import antenv
            antenv.axon_hooks = mod
        except Exception:
            pass


_install_env_fixes()

# ---------------------------------------------------------------------------
# Host preprocessing
# ---------------------------------------------------------------------------

def preprocess(x, edge_index, batch, n_graphs):
    N = x.shape[0]
    GPC = n_graphs // M
    e0 = np.asarray(edge_index[0], np.int64)
    e1 = np.asarray(edge_index[1], np.int64)
    # self-loops are excluded from the gathered edge stream (handled as a
    # cheap identity-matmul term from the SBUF-resident local z); they still
    # count toward the degrees
    src = e0
    dst = e1
    deg = (np.bincount(e1, minlength=N) + 1).astype(np.float32)
    dis = 1.0 / np.sqrt(deg)
    norm = (dis[src] * dis[dst]).astype(np.float32)
    selfnorm = (dis * dis).astype(np.float32)

    batch = np.asarray(batch, np.int64)
    owner = batch // GPC
    node_start = np.searchsorted(batch, np.arange(M) * GPC)
    node_end = np.searchsorted(batch, np.arange(M) * GPC + GPC)
    n_c = node_end - node_start
    T = int(np.ceil(n_c.max() / 128))
    T = ((T + 3) // 4) * 4  # quarter boundaries must align to whole tiles
    S = T * 128
    S4 = S // 4
    assert M * S4 <= 32767, (T, S)
    local = np.arange(N) - node_start[owner]

    eo = owner[dst]
    ld = local[dst]
    et = ld // 128
    ep = (ld % 128).astype(np.float32)
    # bucket = source node's local quarter; each quarter is AllGathered into
    # its own [M*S4, 128] table so layer k+1's descriptor generation for
    # bucket q only waits on quarter-q's collective
    ls = local[src]
    eb = ls // S4
    erel = owner[src] * S4 + (ls - eb * S4)

    # uniform (max-over-cores) padded block counts per (tile, bucket)
    key = ((eo * T + et) * 4 + eb)
    counts = np.bincount(key, minlength=M * T * 4).reshape(M, T, 4)
    nblk_tb = -(-counts.max(axis=0) // 128)  # [T, 4]

    nchunks = -(-T // CH)
    blk_of_tb = np.zeros((T, 4), np.int64)
    chunk_info = []
    call_tb = []  # (t, b) per call, in program order, for the count tensor
    gb = 0
    for k in range(nchunks):
        t0, t1 = k * CH, min((k + 1) * CH, T)
        blk0 = gb
        calls = []
        for b in range(4):
            rblk0 = gb
            for t in range(t0, t1):
                blk_of_tb[t, b] = gb
                gb += int(nblk_tb[t, b])
            nreg = gb - rblk0
            off = 0
            while off < nreg:
                nb = min(nreg - off, CALL_MAX // 128)
                calls.append((b, (rblk0 + off) * 128, nb * 128,
                              (rblk0 + off) - blk0, len(call_tb)))
                call_tb.append(None)
                off += nb
        tiles = []
        for t in range(t0, t1):
            cols = []
            for b in range(4):
                st = int(blk_of_tb[t, b] - blk0)
                cols.extend(range(st, st + int(nblk_tb[t, b])))
            tiles.append((t, cols))
        chunk_info.append(dict(blk0=blk0, nblk=gb - blk0, calls=calls,
                               tiles=tiles))
    total_blocks = gb
    total_slots = gb * 128
    ncalls = len(call_tb)

    # per-core slot arrays
    ordk = np.argsort(key, kind="stable")
    ks = key[ordk]
    group_start = np.zeros(M * T * 4 + 1, np.int64)
    np.cumsum(counts.reshape(-1), out=group_start[1:])
    pos = np.arange(len(ks)) - group_start[ks]
    slot_base = blk_of_tb.reshape(-1)
    slot = slot_base[ks % (T * 4)] * 128 + pos
    core_of = ks // (T * 4)

    er_s = erel[ordk]
    nm_s = norm[ordk]
    ep_s = ep[ordk]

    cnt_g = np.bincount(batch, minlength=n_graphs).astype(np.float32)

    in_maps = []
    for c in range(M):
        selc = core_of == c
        sl = slot[selc]
        idx16 = np.zeros(total_slots, np.int16)
        nrm = np.zeros(total_slots, np.float16)
        lds = np.zeros(total_slots, np.float16)
        idx16[sl] = er_s[selc].astype(np.int16)
        nrm[sl] = nm_s[selc].astype(np.float16)
        lds[sl] = ep_s[selc].astype(np.float16)
        idxw = np.ascontiguousarray(
            np.tile(idx16.reshape(-1, 16).T, (8, 1)))
        nrm2 = np.ascontiguousarray(nrm.reshape(-1, 128).T)
        lds2 = np.ascontiguousarray(lds.reshape(-1, 128).T)

        xT = np.zeros((128, S), np.float16)
        xT[:, :n_c[c]] = x[node_start[c]:node_end[c]].T

        gflat = np.full(T * 128, -1.0, np.float16)
        gflat[:n_c[c]] = (batch[node_start[c]:node_end[c]] - c * GPC)
        gcol = np.ascontiguousarray(gflat.reshape(T, 128).T)

        sn = np.zeros(T * 128, np.float32)
        sn[:n_c[c]] = selfnorm[node_start[c]:node_end[c]]
        snorm = np.ascontiguousarray(sn.reshape(T, 128).T)

        invcnt = np.zeros((128, 1), np.float32)
        invcnt[:GPC, 0] = 1.0 / np.maximum(cnt_g[c * GPC:(c + 1) * GPC], 1.0)

        in_maps.append(dict(idx=idxw, nrm=nrm2, lds=lds2, xT=xT, gcol=gcol,
                            invcnt=invcnt, snorm=snorm))
    meta = dict(T=T, S=S, GPC=GPC, chunks=chunk_info,
                total_blocks=total_blocks, total_slots=total_slots,
                ncalls=ncalls)
    return in_maps, meta


def make_weight_inputs(W1, b1, Wh, bh, Wout, bout):
    Wh = np.asarray(Wh, np.float32)
    iota16 = np.broadcast_to(np.arange(128, dtype=np.float16), (128, 128)).copy()
    ident = np.eye(H, dtype=np.float32)
    ident128 = np.eye(128, dtype=np.float16)
    return dict(
        ident128=ident128,
        W1=np.ascontiguousarray(W1, np.float32).astype(np.float16),
        Wh2=np.ascontiguousarray(
            Wh.transpose(1, 0, 2).reshape(H, 4 * H)).astype(np.float16),
        b14=np.ascontiguousarray(
            np.stack([b1, bh[0], bh[1], bh[2]], axis=1), np.float32),
        b5rep=np.broadcast_to(np.asarray(bh[3], np.float32), (128, H)).copy(),
        woutrep=np.broadcast_to(np.asarray(Wout, np.float32)[:, 0], (128, H)).copy(),
        iota16=iota16,
        ident=ident,
    ), dict(bout=np.asarray(bout, np.float32))


# ---------------------------------------------------------------------------
# Bass program
# ---------------------------------------------------------------------------

def build_nc(meta, weights):
    T, S = meta["T"], meta["S"]
    chunks = meta["chunks"]
    total_blocks = meta["total_blocks"]
    total_slots = meta["total_slots"]
    ncalls = meta["ncalls"]
    nc = bass.Bass("TRN2", target_bir_lowering=False)

    xT_d = nc.declare_dram_parameter("xT", [128, S], F16, isOutput=False)
    idx_d = nc.declare_dram_parameter("idx", [128, total_slots // 16], I16,
                                      isOutput=False)
    nrm_d = nc.declare_dram_parameter("nrm", [128, total_blocks], F16,
                                      isOutput=False)
    lds_d = nc.declare_dram_parameter("lds", [128, total_blocks], F16,
                                      isOutput=False)
    gcol_d = nc.declare_dram_parameter("gcol", [128, T], F16, isOutput=False)
    invcnt_d = nc.declare_dram_parameter("invcnt", [128, 1], F32, isOutput=False)
    iota16_d = nc.declare_dram_parameter("iota16", [128, 128], F16, isOutput=False)
    W1_d = nc.declare_dram_parameter("W1", [128, H], F16, isOutput=False)
    Wh2_d = nc.declare_dram_parameter("Wh2", [H, 4 * H], F16, isOutput=False)
    b14_d = nc.declare_dram_parameter("b14", [H, 4], F32, isOutput=False)
    b5rep_d = nc.declare_dram_parameter("b5rep", [128, H], F32, isOutput=False)
    woutrep_d = nc.declare_dram_parameter("woutrep", [128, H], F32, isOutput=False)
    ident_d = nc.declare_dram_parameter("ident", [H, H], F32, isOutput=False)
    ident128_d = nc.declare_dram_parameter("ident128", [128, 128], F16,
                                           isOutput=False)
    snorm_d = nc.declare_dram_parameter("snorm", [128, T], F32, isOutput=False)
    out_d = nc.declare_dram_parameter("out", [128, 1], F32, isOutput=True)
    bout = float(weights["bout"][0])

    with tile.TileContext(nc) as tc:
        with contextlib.ExitStack() as ctx:
            zfull_pool = ctx.enter_context(
                tc.tile_pool(name="zfull", bufs=1, space="DRAM"))
            dram = ctx.enter_context(tc.tile_pool(name="dram", bufs=1, space="DRAM"))
            const = ctx.enter_context(tc.tile_pool(name="const", bufs=1))
            xp = ctx.enter_context(tc.tile_pool(name="xp", bufs=3))
            gp = ctx.enter_context(tc.tile_pool(name="gp", bufs=2))
            mp = ctx.enter_context(tc.tile_pool(name="mp", bufs=2))
            hp = ctx.enter_context(tc.tile_pool(name="hp", bufs=3))
            zp = ctx.enter_context(tc.tile_pool(name="zp", bufs=3))
            pop = ctx.enter_context(tc.tile_pool(name="pop", bufs=3))
            ep = ctx.enter_context(tc.tile_pool(name="ep", bufs=1))
            ps_agg = ctx.enter_context(tc.tile_pool(name="ps_agg", bufs=2, space="PSUM"))
            ps_z = ctx.enter_context(tc.tile_pool(name="ps_z", bufs=2, space="PSUM"))
            ps_pool = ctx.enter_context(tc.tile_pool(name="ps_pool", bufs=1, space="PSUM"))

            nc.gpsimd.load_library(library_config.mlp)
            # one shared register for dma_gather's num_idxs (to_reg would
            # allocate a fresh register per call and exhaust the file)
            nidx_reg = nc.alloc_register(mybir.EngineType.Pool, "nidx_reg")

            S4 = S // 4
            zfulls = [[zfull_pool.tile([M * S4, 128], F16, addr_space="Shared",
                                       name=f"zfull{k}_{q}", tag=f"zfull{k}_{q}")
                       for q in range(4)]
                      for k in range(5)]
            bounces = [dram.tile([S, 128], F16, name=f"bounce{k}", tag=f"bounce{k}")
                       for k in range(5)]

            idx_sb = const.tile([128, total_slots // 16], I16)
            nc.sync.dma_start(idx_sb[:], idx_d[:])
            nrm_sb = const.tile([128, total_blocks], F16)
            nc.sync.dma_start(nrm_sb[:], nrm_d[:])
            lds_sb = const.tile([128, total_blocks], F16)
            nc.sync.dma_start(lds_sb[:], lds_d[:])
            gcol_sb = const.tile([128, T], F16)
            nc.sync.dma_start(gcol_sb[:], gcol_d[:])
            invcnt_sb = const.tile([128, 1], F32)
            nc.sync.dma_start(invcnt_sb[:], invcnt_d[:])
            iota16_sb = const.tile([128, 128], F16)
            nc.sync.dma_start(iota16_sb[:], iota16_d[:])
            W1_sb = const.tile([128, H], F16)
            nc.sync.dma_start(W1_sb[:], W1_d[:])
            Wh2_sb = const.tile([H, 4 * H], F16)
            nc.sync.dma_start(Wh2_sb[:], Wh2_d[:])
            b14_sb = const.tile([H, 4], F32)
            nc.sync.dma_start(b14_sb[:], b14_d[:])
            b5rep_sb = const.tile([128, H], F32)
            nc.sync.dma_start(b5rep_sb[:], b5rep_d[:])
            woutrep_sb = const.tile([128, H], F32)
            nc.sync.dma_start(woutrep_sb[:], woutrep_d[:])
            ident_sb = const.tile([H, H], F32)
            nc.sync.dma_start(ident_sb[:], ident_d[:])
            ident128_sb = const.tile([128, 128], F16)
            nc.sync.dma_start(ident128_sb[:], ident128_d[:])
            snorm_sb = const.tile([128, T], F32)
            nc.sync.dma_start(snorm_sb[:], snorm_d[:])
            # the core's own z tiles for the current layer (self-loop source)
            zall = const.tile([128, T * H], F16)

            # layer 0: z0 = x @ W1 per local tile, then AllGather
            for t in range(T):
                xt = xp.tile([128, 128], F16, tag="xt")
                nc.sync.dma_start(xt[:], xT_d[:, t * 128:(t + 1) * 128])
                pz = ps_z.tile([128, H], F32, space="PSUM", tag="pz")
                nc.tensor.matmul(out=pz[:], lhsT=xt[:], rhs=W1_sb[:],
                                 start=True, stop=True)
                zt = zp.tile([128, 128], F16, tag="zt")
                nc.scalar.copy(zt[:, :H], pz[:])
                nc.scalar.copy(zall[:, t * H:(t + 1) * H], pz[:])
                nc.sync.dma_start(bounces[0][t * 128:(t + 1) * 128, :], zt[:])
            for q in range(4):
                nc.gpsimd.collective_compute(
                    "AllGather", mybir.AluOpType.bypass,
                    replica_groups=[list(range(M))],
                    ins=[bounces[0][q * S4:(q + 1) * S4, :]],
                    outs=[zfulls[0][q][:]])

            n_pool = sum(1 for ci in chunks for (t, cols) in ci["tiles"])
            pool_ps = None
            pool_i = 0

            for layer in range(1, 6):
                zf = zfulls[layer - 1]
                if layer == 5:
                    pool_ps = ps_pool.tile([H, 128], F32, space="PSUM", tag="pool")
                for ci in chunks:
                    cb = ci["nblk"]
                    blk0 = ci["blk0"]
                    g = gp.tile([128, cb, 128], F16, tag="g")
                    for (b, slot0, nidx, blkoff, call_i) in ci["calls"]:
                        nc.gpsimd.reg_mov(nidx_reg, nidx)
                        nc.gpsimd.dma_gather(
                            g[:, blkoff:blkoff + nidx // 128, :],
                            zf[b][:],
                            idx_sb[:, slot0 // 16:(slot0 + nidx) // 16],
                            nidx, nidx_reg, 128, single_packet=False)
                    gs = g[:, :, 0:H]
                    nrm3 = nrm_sb[:, blk0:blk0 + cb, None].to_broadcast([128, cb, H])
                    nc.vector.tensor_tensor(out=gs, in0=gs, in1=nrm3,
                                            op=mybir.AluOpType.mult)
                    m01 = mp.tile([128, cb, 128], F16, tag="m01")
                    iota3 = iota16_sb[:, None, :].to_broadcast([128, cb, 128])
                    lds3 = lds_sb[:, blk0:blk0 + cb, None].to_broadcast([128, cb, 128])
                    nc.vector.tensor_tensor(out=m01[:], in0=iota3, in1=lds3,
                                            op=mybir.AluOpType.is_equal)
                    for (t, cols) in ci["tiles"]:
                        selfmsg = hp.tile([128, H], F16, tag="selfmsg")
                        nc.vector.tensor_scalar(
                            out=selfmsg[:], in0=zall[:, t * H:(t + 1) * H],
                            scalar1=snorm_sb[:, t:t + 1], scalar2=None,
                            op0=mybir.AluOpType.mult)
                        if layer < 5:
                            pT = ps_agg.tile([H, 128], F32, space="PSUM", tag="pT")
                            for j, col in enumerate(cols):
                                nc.tensor.matmul(
                                    out=pT[:], lhsT=g[:, col, 0:H],
                                    rhs=m01[:, col, :],
                                    start=(j == 0), stop=False)
                            nc.tensor.matmul(
                                out=pT[:], lhsT=selfmsg[:], rhs=ident128_sb[:],
                                start=(len(cols) == 0), stop=True)
                            hT = hp.tile([H, 128], F16, tag="hT")
                            nc.scalar.activation(
                                hT[:], pT[:], mybir.ActivationFunctionType.Relu,
                                bias=b14_sb[:, layer - 1:layer])
                            pz = ps_z.tile([128, H], F32, space="PSUM", tag="pz")
                            nc.tensor.matmul(
                                out=pz[:], lhsT=hT[:],
                                rhs=Wh2_sb[:, (layer - 1) * H:layer * H],
                                start=True, stop=True)
                            zt = zp.tile([128, 128], F16, tag="zt")
                            nc.scalar.copy(zt[:, :H], pz[:])
                            nc.scalar.copy(zall[:, t * H:(t + 1) * H], pz[:])
                            nc.sync.dma_start(
                                bounces[layer][t * 128:(t + 1) * 128, :], zt[:])
                        else:
                            p5 = ps_agg.tile([128, H], F32, space="PSUM", tag="pT")
                            for j, col in enumerate(cols):
                                nc.tensor.matmul(
                                    out=p5[:], lhsT=m01[:, col, :],
                                    rhs=g[:, col, 0:H],
                                    start=(j == 0), stop=False)
                            nc.tensor.matmul(
                                out=p5[:], lhsT=ident128_sb[:], rhs=selfmsg[:],
                                start=(len(cols) == 0), stop=True)
                            h5 = hp.tile([128, H], F16, tag="hT")
                            nc.vector.tensor_tensor(out=h5[:], in0=p5[:],
                                                    in1=b5rep_sb[:],
                                                    op=mybir.AluOpType.add)
                            nc.scalar.activation(h5[:], h5[:],
                                                 mybir.ActivationFunctionType.Relu)
                            pt = pop.tile([128, 128], F16, tag="pt")
                            nc.vector.tensor_tensor(
                                out=pt[:], in0=iota16_sb[:],
                                in1=gcol_sb[:, t:t + 1].to_broadcast([128, 128]),
                                op=mybir.AluOpType.is_equal)
                            nc.tensor.matmul(
                                out=pool_ps[:], lhsT=h5[:], rhs=pt[:],
                                start=(pool_i == 0), stop=(pool_i == n_pool - 1))
                            pool_i += 1
                if layer < 5:
                    for q in range(4):
                        nc.gpsimd.collective_compute(
                            "AllGather", mybir.AluOpType.bypass,
                            replica_groups=[list(range(M))],
                            ins=[bounces[layer][q * S4:(q + 1) * S4, :]],
                            outs=[zfulls[layer][q][:]])

            # transpose, scale by 1/cnt, layernorm, output head (f32 tail)
            pool_sb = ep.tile([H, 128], F32, tag="poolsb")
            nc.scalar.copy(pool_sb[:], pool_ps[:])
            ptr = ps_z.tile([128, H], F32, space="PSUM", tag="pz")
            nc.tensor.transpose(out=ptr[:], in_=pool_sb[:], identity=ident_sb[:])
            pooled = ep.tile([128, H], F32, tag="pooled")
            nc.vector.tensor_scalar(out=pooled[:], in0=ptr[:],
                                    scalar1=invcnt_sb[:, 0:1], scalar2=None,
                                    op0=mybir.AluOpType.mult)
            mu = ep.tile([128, 1], F32, tag="mu")
            nc.vector.tensor_reduce(out=mu[:], in_=pooled[:],
                                    axis=mybir.AxisListType.X,
                                    op=mybir.AluOpType.add)
            nc.vector.tensor_scalar(out=mu[:], in0=mu[:], scalar1=1.0 / H,
                                    scalar2=None, op0=mybir.AluOpType.mult)
            xc = ep.tile([128, H], F32, tag="xc")
            nc.vector.tensor_scalar(out=xc[:], in0=pooled[:], scalar1=mu[:, 0:1],
                                    scalar2=None, op0=mybir.AluOpType.subtract)
            sq = ep.tile([128, H], F32, tag="sq")
            nc.scalar.activation(sq[:], xc[:], mybir.ActivationFunctionType.Square)
            var = ep.tile([128, 1], F32, tag="var")
            nc.vector.tensor_reduce(out=var[:], in_=sq[:],
                                    axis=mybir.AxisListType.X,
                                    op=mybir.AluOpType.add)
            nc.vector.tensor_scalar(out=var[:], in0=var[:], scalar1=1.0 / H,
                                    scalar2=None, op0=mybir.AluOpType.mult)
            eps_col = ep.tile([128, 1], F32, tag="eps")
            nc.vector.memset(eps_col[:], 1e-5)
            std = ep.tile([128, 1], F32, tag="std")
            nc.scalar.activation(std[:], var[:], mybir.ActivationFunctionType.Sqrt,
                                 bias=eps_col[:, 0:1])
            rstd = ep.tile([128, 1], F32, tag="rstd")
            nc.vector.reciprocal(rstd[:], std[:])
            ln = ep.tile([128, H], F32, tag="ln")
            nc.vector.tensor_scalar(out=ln[:], in0=xc[:], scalar1=rstd[:, 0:1],
                                    scalar2=None, op0=mybir.AluOpType.mult)
            y = ep.tile([128, H], F32, tag="y")
            nc.vector.tensor_tensor(out=y[:], in0=ln[:], in1=woutrep_sb[:],
                                    op=mybir.AluOpType.mult)
            yr = ep.tile([128, 1], F32, tag="yr")
            nc.vector.tensor_reduce(out=yr[:], in_=y[:], axis=mybir.AxisListType.X,
                                    op=mybir.AluOpType.add)
            nc.vector.tensor_scalar(out=yr[:], in0=yr[:], scalar1=bout,
                                    scalar2=None, op0=mybir.AluOpType.add)
            nc.sync.dma_start(out_d[:], yr[:])

    from concourse.library_overlay import lower_extended_insts
    lower_extended_insts(nc)
    return nc


# ---------------------------------------------------------------------------
# Entry point
# ---------------------------------------------------------------------------

def kernel(x, edge_index, batch, W1, b1, Wh, bh, Wout, bout):
    from concourse.bass_utils import run_bass_kernel_spmd

    x = np.asarray(x, np.float32)
    edge_index = np.asarray(edge_index)
    batch = np.asarray(batch)
    n_graphs = 1000

    in_maps, meta = preprocess(x, edge_index, batch, n_graphs)
    wmaps, wmeta = make_weight_inputs(W1, b1, Wh, bh, Wout, bout)
    nc = build_nc(meta, dict(bout=wmeta["bout"]))
    for im in in_maps:
        im.update(wmaps)

    import time
    last_err = None
    for attempt in range(3):
        try:
            res = run_bass_kernel_spmd(nc, in_maps, core_ids=list(range(M)))
            break
        except Exception as e:  # transient terminal hiccups / device recovery
            last_err = e
            time.sleep(30 * (attempt + 1))
    else:
        raise last_err

    GPC = meta["GPC"]
    out = np.concatenate([res.results[c]["out"][:GPC] for c in range(M)], axis=0)
    return np.ascontiguousarray(out, np.float32)


# revision 33
# speedup vs baseline: 1.6199x; 1.0173x over previous
"""GCN-5 message-passing kernel for Trainium2, 8-core SPMD Bass/Tile.

Strategy (graph-parallel per the sharding hint), v2:
  - batch is sorted, so graphs are contiguous node ranges.  Core c owns graphs
    [125c, 125(c+1)) and their nodes; edges are assigned to the core owning the
    dst node.  Pooling / layernorm / output head are fully local per core.
  - Everything on the edge path is fp16 (tolerance is 2e-2): z rows are stored
    padded to 128 fp16 (256B) so one gather descriptor moves a full row.
  - Per layer: z = h @ W per local tile on the TensorEngine (fp16, 1 cyc/row
    vs fp32's 4), AllGather (fp16, same bytes as the old f32x64), then batched
    `dma_gather` (InstDMAGatherAnt, ~1us fixed + 0.34ns/row) replaces the old
    per-128-edge indirect_dma_start (~1.2us each) -- descriptor generation on
    GPSIMD drops from ~7ms to ~0.6ms.
  - dma_gather indices are int16, so the 101376-row z table is bucketed into
    4 ranges of 2 cores (2*S <= 32767 rows); edges are grouped per
    (tile-chunk, bucket) with uniform-across-cores padded block counts (the
    SPMD program is shared, so counts are max'd over cores; pads carry idx=0
    and norm=0).  Calls are capped at 4096 idxs with single_packet=False
    (larger/packed calls wedge the SWDGE ring on this runtime).
  - Scatter-add per 128-dst tile stays a one-hot matmul (fp16) with PSUM
    accumulation; the one-hot is built per chunk in one DVE is_equal.
  - Layer 5 aggregates in [node, H] orientation and feeds the per-graph
    mean-pool one-hot matmul directly, accumulating all tiles in one PSUM
    bank; layernorm + head are unchanged f32.
"""
import sys
import types
import contextlib

import numpy as np

sys.path.insert(0, "/opt/trn_rl_repo")

import concourse.bass as bass
import concourse.tile as tile
from concourse import mybir, library_config
from concourse.vector_clock import ScopedClock

F32 = mybir.dt.float32
F16 = mybir.dt.float16
I16 = mybir.dt.int16
M = 8  # NeuronCores
H = 64
CH = 8          # tiles per gather chunk
CALL_MAX = 4096  # max idxs per dma_gather call

# ---------------------------------------------------------------------------
# Environment fixes for this container
# ---------------------------------------------------------------------------

def _install_env_fixes():
    import concourse.tile as tile_mod

    def _patched_drain_and_barrier(self, tick_clock, wait_clock):
        # this walrus build allows a single sync-wait per TPB_CTRL Drain;
        # split the Tile tail-drain's waits across multiple drains.
        nc = self.nc
        drain_inst = nc.sync.drain()
        wait_clock.add_sem_waits(drain_inst.ins,
                                 ScopedClock({None: tick_clock.global_clock}))
        si = drain_inst.ins.sync_info
        waits = list(si.on_wait or [])
        if len(waits) > 1:
            si.on_wait[:] = waits[:1]
            for w in waits[1:]:
                d2 = nc.sync.drain()
                if d2.ins.sync_info is None:
                    d2.ins.sync_info = mybir.SyncInfo(on_wait=[w], on_update=[])
                else:
                    d2.ins.sync_info.on_wait.append(w)
        nc.all_engine_barrier()
        assert self.sems is not None
        popped = nc._tile_sem_poison_stack.pop()
        assert popped is self._sem_poison
        nc.clear_and_free_semaphores(list(self.sems.allocated().values()))
        nc.all_engine_barrier()

    tile_mod.TileContext._drain_and_barrier = _patched_drain_and_barrier

    _orig_lower_ordered = tile_mod.TileContext._lower_ordered_insts

    def _split_multiwait_lower(self, ordered):
        nc = self.nc
        for bbname, insts in ordered.items():
            newlist = []
            changed = False
            for inst in insts:
                si = getattr(inst, "sync_info", None)
                eng = getattr(inst, "engine", None)
                if (si is not None and si.on_wait and len(si.on_wait) > 1
                        and eng is not None and eng != mybir.EngineType.Unassigned
                        and inst.is_executable()):
                    waits = list(si.on_wait)
                    si.on_wait[:] = waits[-1:]
                    for w in waits[:-1]:
                        nop = mybir.InstNoOp(
                            name=nc.get_next_instruction_name(), engine=eng)
                        nop.sync_info = mybir.SyncInfo(on_wait=[w], on_update=[])
                        try:
                            nc.register_instruction(nop, overwrite=True)
                        except Exception:
                            pass
                        newlist.append(nop)
                    changed = True
                newlist.append(inst)
            if changed:
                insts[:] = newlist
        return _orig_lower_ordered(self, ordered)

    if getattr(tile_mod.TileContext._lower_ordered_insts, "__name__", "") != \
            "_split_multiwait_lower":
        tile_mod.TileContext._lower_ordered_insts = _split_multiwait_lower

    # NTFF profile hook (lets trace=True work under axon); best-effort.
    if "antenv.axon_hooks" not in sys.modules:
        try:
            from trn_agent_boot.trn_boot import _ntff_profile_via_ctypes
            hook = _ntff_profile_via_ctypes("/opt/axon/libaxon_pjrt.so")
            mod = types.ModuleType("antenv.axon_hooks")
            mod.get_axon_ntff_profile_hook = lambda: hook
            mod.set_axon_ntff_profile_hook = lambda h: None
            sys.modules["antenv.axon_hooks"] = mod
            import antenv
            antenv.axon_hooks = mod
        except Exception:
            pass


_install_env_fixes()

# ---------------------------------------------------------------------------
# Host preprocessing
# ---------------------------------------------------------------------------

def preprocess(x, edge_index, batch, n_graphs):
    N = x.shape[0]
    GPC = n_graphs // M
    e0 = np.asarray(edge_index[0], np.int64)
    e1 = np.asarray(edge_index[1], np.int64)
    # self-loops are excluded from the gathered edge stream (handled as a
    # cheap identity-matmul term from the SBUF-resident local z); they still
    # count toward the degrees
    src = e0
    dst = e1
    deg = (np.bincount(e1, minlength=N) + 1).astype(np.float32)
    dis = 1.0 / np.sqrt(deg)
    norm = (dis[src] * dis[dst]).astype(np.float32)
    selfnorm = (dis * dis).astype(np.float32)

    batch = np.asarray(batch, np.int64)
    owner = batch // GPC
    node_start = np.searchsorted(batch, np.arange(M) * GPC)
    node_end = np.searchsorted(batch, np.arange(M) * GPC + GPC)
    n_c = node_end - node_start
    T = int(np.ceil(n_c.max() / 128))
    T = ((T + 3) // 4) * 4  # quarter boundaries must align to whole tiles
    S = T * 128
    S4 = S // 4
    assert M * S4 <= 32767, (T, S)
    local = np.arange(N) - node_start[owner]

    eo = owner[dst]
    ld = local[dst]
    et = ld // 128
    ep = (ld % 128).astype(np.float32)
    # bucket = source node's local quarter; each quarter is AllGathered into
    # its own [M*S4, 128] table so layer k+1's descriptor generation for
    # bucket q only waits on quarter-q's collective
    ls = local[src]
    eb = ls // S4
    erel = owner[src] * S4 + (ls - eb * S4)

    # uniform (max-over-cores) padded block counts per (tile, bucket)
    key = ((eo * T + et) * 4 + eb)
    counts = np.bincount(key, minlength=M * T * 4).reshape(M, T, 4)
    nblk_tb = -(-counts.max(axis=0) // 128)  # [T, 4]

    nchunks = -(-T // CH)
    blk_of_tb = np.zeros((T, 4), np.int64)
    chunk_info = []
    call_tb = []  # (t, b) per call, in program order, for the count tensor
    gb = 0
    for k in range(nchunks):
        t0, t1 = k * CH, min((k + 1) * CH, T)
        blk0 = gb
        calls = []
        for b in range(4):
            rblk0 = gb
            for t in range(t0, t1):
                blk_of_tb[t, b] = gb
                gb += int(nblk_tb[t, b])
            nreg = gb - rblk0
            off = 0
            while off < nreg:
                nb = min(nreg - off, CALL_MAX // 128)
                calls.append((b, (rblk0 + off) * 128, nb * 128,
                              (rblk0 + off) - blk0, len(call_tb)))
                call_tb.append(None)
                off += nb
        tiles = []
        for t in range(t0, t1):
            cols = []
            for b in range(4):
                st = int(blk_of_tb[t, b] - blk0)
                cols.extend(range(st, st + int(nblk_tb[t, b])))
            tiles.append((t, cols))
        chunk_info.append(dict(blk0=blk0, nblk=gb - blk0, calls=calls,
                               tiles=tiles))
    total_blocks = gb
    total_slots = gb * 128
    ncalls = len(call_tb)

    # per-core slot arrays
    ordk = np.argsort(key, kind="stable")
    ks = key[ordk]
    group_start = np.zeros(M * T * 4 + 1, np.int64)
    np.cumsum(counts.reshape(-1), out=group_start[1:])
    pos = np.arange(len(ks)) - group_start[ks]
    slot_base = blk_of_tb.reshape(-1)
    slot = slot_base[ks % (T * 4)] * 128 + pos
    core_of = ks // (T * 4)

    er_s = erel[ordk]
    nm_s = norm[ordk]
    ep_s = ep[ordk]

    cnt_g = np.bincount(batch, minlength=n_graphs).astype(np.float32)

    in_maps = []
    for c in range(M):
        selc = core_of == c
        sl = slot[selc]
        idx16 = np.zeros(total_slots, np.int16)
        nrm = np.zeros(total_slots, np.float16)
        lds = np.zeros(total_slots, np.float16)
        idx16[sl] = er_s[selc].astype(np.int16)
        nrm[sl] = nm_s[selc].astype(np.float16)
        lds[sl] = ep_s[selc].astype(np.float16)
        idxw = np.ascontiguousarray(
            np.tile(idx16.reshape(-1, 16).T, (8, 1)))
        nrm2 = np.ascontiguousarray(nrm.reshape(-1, 128).T)
        lds2 = np.ascontiguousarray(lds.reshape(-1, 128).T)

        xT = np.zeros((128, S), np.float16)
        xT[:, :n_c[c]] = x[node_start[c]:node_end[c]].T

        gflat = np.full(T * 128, -1.0, np.float16)
        gflat[:n_c[c]] = (batch[node_start[c]:node_end[c]] - c * GPC)
        gcol = np.ascontiguousarray(gflat.reshape(T, 128).T)

        sn = np.zeros(T * 128, np.float32)
        sn[:n_c[c]] = selfnorm[node_start[c]:node_end[c]]
        snorm = np.ascontiguousarray(sn.reshape(T, 128).T)

        invcnt = np.zeros((128, 1), np.float32)
        invcnt[:GPC, 0] = 1.0 / np.maximum(cnt_g[c * GPC:(c + 1) * GPC], 1.0)

        in_maps.append(dict(idx=idxw, nrm=nrm2, lds=lds2, xT=xT, gcol=gcol,
                            invcnt=invcnt, snorm=snorm))
    meta = dict(T=T, S=S, GPC=GPC, chunks=chunk_info,
                total_blocks=total_blocks, total_slots=total_slots,
                ncalls=ncalls)
    return in_maps, meta


def make_weight_inputs(W1, b1, Wh, bh, Wout, bout):
    Wh = np.asarray(Wh, np.float32)
    iota16 = np.broadcast_to(np.arange(128, dtype=np.float16), (128, 128)).copy()
    ident = np.eye(H, dtype=np.float32)
    ident128 = np.eye(128, dtype=np.float16)
    return dict(
        ident128=ident128,
        W1=np.ascontiguousarray(W1, np.float32).astype(np.float16),
        Wh2=np.ascontiguousarray(
            Wh.transpose(1, 0, 2).reshape(H, 4 * H)).astype(np.float16),
        b14=np.ascontiguousarray(
            np.stack([b1, bh[0], bh[1], bh[2]], axis=1), np.float32),
        b5rep=np.broadcast_to(np.asarray(bh[3], np.float32), (128, H)).copy(),
        woutrep=np.broadcast_to(np.asarray(Wout, np.float32)[:, 0], (128, H)).copy(),
        iota16=iota16,
        ident=ident,
    ), dict(bout=np.asarray(bout, np.float32))


# ---------------------------------------------------------------------------
# Bass program
# ---------------------------------------------------------------------------

def build_nc(meta, weights):
    T, S = meta["T"], meta["S"]
    chunks = meta["chunks"]
    total_blocks = meta["total_blocks"]
    total_slots = meta["total_slots"]
    ncalls = meta["ncalls"]
    nc = bass.Bass("TRN2", target_bir_lowering=False)

    xT_d = nc.declare_dram_parameter("xT", [128, S], F16, isOutput=False)
    idx_d = nc.declare_dram_parameter("idx", [128, total_slots // 16], I16,
                                      isOutput=False)
    nrm_d = nc.declare_dram_parameter("nrm", [128, total_blocks], F16,
                                      isOutput=False)
    lds_d = nc.declare_dram_parameter("lds", [128, total_blocks], F16,
                                      isOutput=False)
    gcol_d = nc.declare_dram_parameter("gcol", [128, T], F16, isOutput=False)
    invcnt_d = nc.declare_dram_parameter("invcnt", [128, 1], F32, isOutput=False)
    iota16_d = nc.declare_dram_parameter("iota16", [128, 128], F16, isOutput=False)
    W1_d = nc.declare_dram_parameter("W1", [128, H], F16, isOutput=False)
    Wh2_d = nc.declare_dram_parameter("Wh2", [H, 4 * H], F16, isOutput=False)
    b14_d = nc.declare_dram_parameter("b14", [H, 4], F32, isOutput=False)
    b5rep_d = nc.declare_dram_parameter("b5rep", [128, H], F32, isOutput=False)
    woutrep_d = nc.declare_dram_parameter("woutrep", [128, H], F32, isOutput=False)
    ident_d = nc.declare_dram_parameter("ident", [H, H], F32, isOutput=False)
    ident128_d = nc.declare_dram_parameter("ident128", [128, 128], F16,
                                           isOutput=False)
    snorm_d = nc.declare_dram_parameter("snorm", [128, T], F32, isOutput=False)
    out_d = nc.declare_dram_parameter("out", [128, 1], F32, isOutput=True)
    bout = float(weights["bout"][0])

    with tile.TileContext(nc) as tc:
        with contextlib.ExitStack() as ctx:
            zfull_pool = ctx.enter_context(
                tc.tile_pool(name="zfull", bufs=1, space="DRAM"))
            dram = ctx.enter_context(tc.tile_pool(name="dram", bufs=1, space="DRAM"))
            const = ctx.enter_context(tc.tile_pool(name="const", bufs=1))
            xp = ctx.enter_context(tc.tile_pool(name="xp", bufs=3))
            gp = ctx.enter_context(tc.tile_pool(name="gp", bufs=3))
            mp = ctx.enter_context(tc.tile_pool(name="mp", bufs=2))
            hp = ctx.enter_context(tc.tile_pool(name="hp", bufs=3))
            zp = ctx.enter_context(tc.tile_pool(name="zp", bufs=3))
            pop = ctx.enter_context(tc.tile_pool(name="pop", bufs=3))
            ep = ctx.enter_context(tc.tile_pool(name="ep", bufs=1))
            ps_agg = ctx.enter_context(tc.tile_pool(name="ps_agg", bufs=2, space="PSUM"))
            ps_z = ctx.enter_context(tc.tile_pool(name="ps_z", bufs=2, space="PSUM"))
            ps_pool = ctx.enter_context(tc.tile_pool(name="ps_pool", bufs=1, space="PSUM"))

            nc.gpsimd.load_library(library_config.mlp)
            # one shared register for dma_gather's num_idxs (to_reg would
            # allocate a fresh register per call and exhaust the file)
            nidx_reg = nc.alloc_register(mybir.EngineType.Pool, "nidx_reg")

            S4 = S // 4
            zfulls = [[zfull_pool.tile([M * S4, 128], F16, addr_space="Shared",
                                       name=f"zfull{k}_{q}", tag=f"zfull{k}_{q}")
                       for q in range(4)]
                      for k in range(5)]
            bounces = [dram.tile([S, 128], F16, name=f"bounce{k}", tag=f"bounce{k}")
                       for k in range(5)]

            idx_sb = const.tile([128, total_slots // 16], I16)
            nc.sync.dma_start(idx_sb[:], idx_d[:])
            nrm_sb = const.tile([128, total_blocks], F16)
            nc.sync.dma_start(nrm_sb[:], nrm_d[:])
            lds_sb = const.tile([128, total_blocks], F16)
            nc.sync.dma_start(lds_sb[:], lds_d[:])
            gcol_sb = const.tile([128, T], F16)
            nc.sync.dma_start(gcol_sb[:], gcol_d[:])
            invcnt_sb = const.tile([128, 1], F32)
            nc.sync.dma_start(invcnt_sb[:], invcnt_d[:])
            iota16_sb = const.tile([128, 128], F16)
            nc.sync.dma_start(iota16_sb[:], iota16_d[:])
            W1_sb = const.tile([128, H], F16)
            nc.sync.dma_start(W1_sb[:], W1_d[:])
            Wh2_sb = const.tile([H, 4 * H], F16)
            nc.sync.dma_start(Wh2_sb[:], Wh2_d[:])
            b14_sb = const.tile([H, 4], F32)
            nc.sync.dma_start(b14_sb[:], b14_d[:])
            b5rep_sb = const.tile([128, H], F32)
            nc.sync.dma_start(b5rep_sb[:], b5rep_d[:])
            woutrep_sb = const.tile([128, H], F32)
            nc.sync.dma_start(woutrep_sb[:], woutrep_d[:])
            ident_sb = const.tile([H, H], F32)
            nc.sync.dma_start(ident_sb[:], ident_d[:])
            ident128_sb = const.tile([128, 128], F16)
            nc.sync.dma_start(ident128_sb[:], ident128_d[:])
            snorm_sb = const.tile([128, T], F32)
            nc.sync.dma_start(snorm_sb[:], snorm_d[:])
            # the core's own z tiles for the current layer (self-loop source)
            zall = const.tile([128, T * H], F16)

            # layer 0: z0 = x @ W1 per local tile, then AllGather
            for t in range(T):
                xt = xp.tile([128, 128], F16, tag="xt")
                nc.sync.dma_start(xt[:], xT_d[:, t * 128:(t + 1) * 128])
                pz = ps_z.tile([128, H], F32, space="PSUM", tag="pz")
                nc.tensor.matmul(out=pz[:], lhsT=xt[:], rhs=W1_sb[:],
                                 start=True, stop=True)
                zt = zp.tile([128, 128], F16, tag="zt")
                nc.scalar.copy(zt[:, :H], pz[:])
                nc.scalar.copy(zall[:, t * H:(t + 1) * H], pz[:])
                nc.sync.dma_start(bounces[0][t * 128:(t + 1) * 128, :], zt[:])
            for q in range(4):
                nc.gpsimd.collective_compute(
                    "AllGather", mybir.AluOpType.bypass,
                    replica_groups=[list(range(M))],
                    ins=[bounces[0][q * S4:(q + 1) * S4, :]],
                    outs=[zfulls[0][q][:]])

            n_pool = sum(1 for ci in chunks for (t, cols) in ci["tiles"])
            pool_ps = None
            pool_i = 0

            for layer in range(1, 6):
                zf = zfulls[layer - 1]
                if layer == 5:
                    pool_ps = ps_pool.tile([H, 128], F32, space="PSUM", tag="pool")
                for ci in chunks:
                    cb = ci["nblk"]
                    blk0 = ci["blk0"]
                    g = gp.tile([128, cb, 128], F16, tag="g")
                    for (b, slot0, nidx, blkoff, call_i) in ci["calls"]:
                        nc.gpsimd.reg_mov(nidx_reg, nidx)
                        nc.gpsimd.dma_gather(
                            g[:, blkoff:blkoff + nidx // 128, :],
                            zf[b][:],
                            idx_sb[:, slot0 // 16:(slot0 + nidx) // 16],
                            nidx, nidx_reg, 128, single_packet=False)
                    gs = g[:, :, 0:H]
                    nrm3 = nrm_sb[:, blk0:blk0 + cb, None].to_broadcast([128, cb, H])
                    nc.vector.tensor_tensor(out=gs, in0=gs, in1=nrm3,
                                            op=mybir.AluOpType.mult)
                    m01 = mp.tile([128, cb, 128], F16, tag="m01")
                    iota3 = iota16_sb[:, None, :].to_broadcast([128, cb, 128])
                    lds3 = lds_sb[:, blk0:blk0 + cb, None].to_broadcast([128, cb, 128])
                    nc.vector.tensor_tensor(out=m01[:], in0=iota3, in1=lds3,
                                            op=mybir.AluOpType.is_equal)
                    for (t, cols) in ci["tiles"]:
                        selfmsg = hp.tile([128, H], F16, tag="selfmsg")
                        nc.vector.tensor_scalar(
                            out=selfmsg[:], in0=zall[:, t * H:(t + 1) * H],
                            scalar1=snorm_sb[:, t:t + 1], scalar2=None,
                            op0=mybir.AluOpType.mult)
                        if layer < 5:
                            pT = ps_agg.tile([H, 128], F32, space="PSUM", tag="pT")
                            for j, col in enumerate(cols):
                                nc.tensor.matmul(
                                    out=pT[:], lhsT=g[:, col, 0:H],
                                    rhs=m01[:, col, :],
                                    start=(j == 0), stop=False)
                            nc.tensor.matmul(
                                out=pT[:], lhsT=selfmsg[:], rhs=ident128_sb[:],
                                start=(len(cols) == 0), stop=True)
                            hT = hp.tile([H, 128], F16, tag="hT")
                            nc.scalar.activation(
                                hT[:], pT[:], mybir.ActivationFunctionType.Relu,
                                bias=b14_sb[:, layer - 1:layer])
                            pz = ps_z.tile([128, H], F32, space="PSUM", tag="pz")
                            nc.tensor.matmul(
                                out=pz[:], lhsT=hT[:],
                                rhs=Wh2_sb[:, (layer - 1) * H:layer * H],
                                start=True, stop=True)
                            zt = zp.tile([128, 128], F16, tag="zt")
                            nc.scalar.copy(zt[:, :H], pz[:])
                            nc.scalar.copy(zall[:, t * H:(t + 1) * H], pz[:])
                            nc.sync.dma_start(
                                bounces[layer][t * 128:(t + 1) * 128, :], zt[:])
                        else:
                            p5 = ps_agg.tile([128, H], F32, space="PSUM", tag="pT")
                            for j, col in enumerate(cols):
                                nc.tensor.matmul(
                                    out=p5[:], lhsT=m01[:, col, :],
                                    rhs=g[:, col, 0:H],
                                    start=(j == 0), stop=False)
                            nc.tensor.matmul(
                                out=p5[:], lhsT=ident128_sb[:], rhs=selfmsg[:],
                                start=(len(cols) == 0), stop=True)
                            h5 = hp.tile([128, H], F16, tag="hT")
                            nc.vector.tensor_tensor(out=h5[:], in0=p5[:],
                                                    in1=b5rep_sb[:],
                                                    op=mybir.AluOpType.add)
                            nc.scalar.activation(h5[:], h5[:],
                                                 mybir.ActivationFunctionType.Relu)
                            pt = pop.tile([128, 128], F16, tag="pt")
                            nc.vector.tensor_tensor(
                                out=pt[:], in0=iota16_sb[:],
                                in1=gcol_sb[:, t:t + 1].to_broadcast([128, 128]),
                                op=mybir.AluOpType.is_equal)
                            nc.tensor.matmul(
                                out=pool_ps[:], lhsT=h5[:], rhs=pt[:],
                                start=(pool_i == 0), stop=(pool_i == n_pool - 1))
                            pool_i += 1
                if layer < 5:
                    for q in range(4):
                        nc.gpsimd.collective_compute(
                            "AllGather", mybir.AluOpType.bypass,
                            replica_groups=[list(range(M))],
                            ins=[bounces[layer][q * S4:(q + 1) * S4, :]],
                            outs=[zfulls[layer][q][:]])

            # transpose, scale by 1/cnt, layernorm, output head (f32 tail)
            pool_sb = ep.tile([H, 128], F32, tag="poolsb")
            nc.scalar.copy(pool_sb[:], pool_ps[:])
            ptr = ps_z.tile([128, H], F32, space="PSUM", tag="pz")
            nc.tensor.transpose(out=ptr[:], in_=pool_sb[:], identity=ident_sb[:])
            pooled = ep.tile([128, H], F32, tag="pooled")
            nc.vector.tensor_scalar(out=pooled[:], in0=ptr[:],
                                    scalar1=invcnt_sb[:, 0:1], scalar2=None,
                                    op0=mybir.AluOpType.mult)
            mu = ep.tile([128, 1], F32, tag="mu")
            nc.vector.tensor_reduce(out=mu[:], in_=pooled[:],
                                    axis=mybir.AxisListType.X,
                                    op=mybir.AluOpType.add)
            nc.vector.tensor_scalar(out=mu[:], in0=mu[:], scalar1=1.0 / H,
                                    scalar2=None, op0=mybir.AluOpType.mult)
            xc = ep.tile([128, H], F32, tag="xc")
            nc.vector.tensor_scalar(out=xc[:], in0=pooled[:], scalar1=mu[:, 0:1],
                                    scalar2=None, op0=mybir.AluOpType.subtract)
            sq = ep.tile([128, H], F32, tag="sq")
            nc.scalar.activation(sq[:], xc[:], mybir.ActivationFunctionType.Square)
            var = ep.tile([128, 1], F32, tag="var")
            nc.vector.tensor_reduce(out=var[:], in_=sq[:],
                                    axis=mybir.AxisListType.X,
                                    op=mybir.AluOpType.add)
            nc.vector.tensor_scalar(out=var[:], in0=var[:], scalar1=1.0 / H,
                                    scalar2=None, op0=mybir.AluOpType.mult)
            eps_col = ep.tile([128, 1], F32, tag="eps")
            nc.vector.memset(eps_col[:], 1e-5)
            std = ep.tile([128, 1], F32, tag="std")
            nc.scalar.activation(std[:], var[:], mybir.ActivationFunctionType.Sqrt,
                                 bias=eps_col[:, 0:1])
            rstd = ep.tile([128, 1], F32, tag="rstd")
            nc.vector.reciprocal(rstd[:], std[:])
            ln = ep.tile([128, H], F32, tag="ln")
            nc.vector.tensor_scalar(out=ln[:], in0=xc[:], scalar1=rstd[:, 0:1],
                                    scalar2=None, op0=mybir.AluOpType.mult)
            y = ep.tile([128, H], F32, tag="y")
            nc.vector.tensor_tensor(out=y[:], in0=ln[:], in1=woutrep_sb[:],
                                    op=mybir.AluOpType.mult)
            yr = ep.tile([128, 1], F32, tag="yr")
            nc.vector.tensor_reduce(out=yr[:], in_=y[:], axis=mybir.AxisListType.X,
                                    op=mybir.AluOpType.add)
            nc.vector.tensor_scalar(out=yr[:], in0=yr[:], scalar1=bout,
                                    scalar2=None, op0=mybir.AluOpType.add)
            nc.sync.dma_start(out_d[:], yr[:])

    from concourse.library_overlay import lower_extended_insts
    lower_extended_insts(nc)
    return nc


# ---------------------------------------------------------------------------
# Entry point
# ---------------------------------------------------------------------------

def kernel(x, edge_index, batch, W1, b1, Wh, bh, Wout, bout):
    from concourse.bass_utils import run_bass_kernel_spmd

    x = np.asarray(x, np.float32)
    edge_index = np.asarray(edge_index)
    batch = np.asarray(batch)
    n_graphs = 1000

    in_maps, meta = preprocess(x, edge_index, batch, n_graphs)
    wmaps, wmeta = make_weight_inputs(W1, b1, Wh, bh, Wout, bout)
    nc = build_nc(meta, dict(bout=wmeta["bout"]))
    for im in in_maps:
        im.update(wmaps)

    import time
    last_err = None
    for attempt in range(3):
        try:
            res = run_bass_kernel_spmd(nc, in_maps, core_ids=list(range(M)))
            break
        except Exception as e:  # transient terminal hiccups / device recovery
            last_err = e
            time.sleep(30 * (attempt + 1))
    else:
        raise last_err

    GPC = meta["GPC"]
    out = np.concatenate([res.results[c]["out"][:GPC] for c in range(M)], axis=0)
    return np.ascontiguousarray(out, np.float32)


# revision 34
# speedup vs baseline: 1.6740x; 1.0334x over previous
"""GCN-5 message-passing kernel for Trainium2, 8-core SPMD Bass/Tile.

Strategy (graph-parallel per the sharding hint), v2:
  - batch is sorted, so graphs are contiguous node ranges.  Core c owns graphs
    [125c, 125(c+1)) and their nodes; edges are assigned to the core owning the
    dst node.  Pooling / layernorm / output head are fully local per core.
  - Everything on the edge path is fp16 (tolerance is 2e-2): z rows are stored
    padded to 128 fp16 (256B) so one gather descriptor moves a full row.
  - Per layer: z = h @ W per local tile on the TensorEngine (fp16, 1 cyc/row
    vs fp32's 4), AllGather (fp16, same bytes as the old f32x64), then batched
    `dma_gather` (InstDMAGatherAnt, ~1us fixed + 0.34ns/row) replaces the old
    per-128-edge indirect_dma_start (~1.2us each) -- descriptor generation on
    GPSIMD drops from ~7ms to ~0.6ms.
  - dma_gather indices are int16, so the 101376-row z table is bucketed into
    4 ranges of 2 cores (2*S <= 32767 rows); edges are grouped per
    (tile-chunk, bucket) with uniform-across-cores padded block counts (the
    SPMD program is shared, so counts are max'd over cores; pads carry idx=0
    and norm=0).  Calls are capped at 4096 idxs with single_packet=False
    (larger/packed calls wedge the SWDGE ring on this runtime).
  - Scatter-add per 128-dst tile stays a one-hot matmul (fp16) with PSUM
    accumulation; the one-hot is built per chunk in one DVE is_equal.
  - Layer 5 aggregates in [node, H] orientation and feeds the per-graph
    mean-pool one-hot matmul directly, accumulating all tiles in one PSUM
    bank; layernorm + head are unchanged f32.
"""
import sys
import types
import contextlib

import numpy as np

sys.path.insert(0, "/opt/trn_rl_repo")

import concourse.bass as bass
import concourse.tile as tile
from concourse import mybir, library_config
from concourse.vector_clock import ScopedClock

F32 = mybir.dt.float32
F16 = mybir.dt.float16
I16 = mybir.dt.int16
M = 8  # NeuronCores
H = 64
CH = 8          # tiles per gather chunk
CALL_MAX = 4096  # max idxs per dma_gather call

# ---------------------------------------------------------------------------
# Environment fixes for this container
# ---------------------------------------------------------------------------

def _install_env_fixes():
    import concourse.tile as tile_mod

    def _patched_drain_and_barrier(self, tick_clock, wait_clock):
        # this walrus build allows a single sync-wait per TPB_CTRL Drain;
        # split the Tile tail-drain's waits across multiple drains.
        nc = self.nc
        drain_inst = nc.sync.drain()
        wait_clock.add_sem_waits(drain_inst.ins,
                                 ScopedClock({None: tick_clock.global_clock}))
        si = drain_inst.ins.sync_info
        waits = list(si.on_wait or [])
        if len(waits) > 1:
            si.on_wait[:] = waits[:1]
            for w in waits[1:]:
                d2 = nc.sync.drain()
                if d2.ins.sync_info is None:
                    d2.ins.sync_info = mybir.SyncInfo(on_wait=[w], on_update=[])
                else:
                    d2.ins.sync_info.on_wait.append(w)
        nc.all_engine_barrier()
        assert self.sems is not None
        popped = nc._tile_sem_poison_stack.pop()
        assert popped is self._sem_poison
        nc.clear_and_free_semaphores(list(self.sems.allocated().values()))
        nc.all_engine_barrier()

    tile_mod.TileContext._drain_and_barrier = _patched_drain_and_barrier

    _orig_lower_ordered = tile_mod.TileContext._lower_ordered_insts

    def _split_multiwait_lower(self, ordered):
        nc = self.nc
        for bbname, insts in ordered.items():
            newlist = []
            changed = False
            for inst in insts:
                si = getattr(inst, "sync_info", None)
                eng = getattr(inst, "engine", None)
                if (si is not None and si.on_wait and len(si.on_wait) > 1
                        and eng is not None and eng != mybir.EngineType.Unassigned
                        and inst.is_executable()):
                    waits = list(si.on_wait)
                    si.on_wait[:] = waits[-1:]
                    for w in waits[:-1]:
                        nop = mybir.InstNoOp(
                            name=nc.get_next_instruction_name(), engine=eng)
                        nop.sync_info = mybir.SyncInfo(on_wait=[w], on_update=[])
                        try:
                            nc.register_instruction(nop, overwrite=True)
                        except Exception:
                            pass
                        newlist.append(nop)
                    changed = True
                newlist.append(inst)
            if changed:
                insts[:] = newlist
        return _orig_lower_ordered(self, ordered)

    if getattr(tile_mod.TileContext._lower_ordered_insts, "__name__", "") != \
            "_split_multiwait_lower":
        tile_mod.TileContext._lower_ordered_insts = _split_multiwait_lower

    # NTFF profile hook (lets trace=True work under axon); best-effort.
    if "antenv.axon_hooks" not in sys.modules:
        try:
            from trn_agent_boot.trn_boot import _ntff_profile_via_ctypes
            hook = _ntff_profile_via_ctypes("/opt/axon/libaxon_pjrt.so")
            mod = types.ModuleType("antenv.axon_hooks")
            mod.get_axon_ntff_profile_hook = lambda: hook
            mod.set_axon_ntff_profile_hook = lambda h: None
            sys.modules["antenv.axon_hooks"] = mod
            import antenv
            antenv.axon_hooks = mod
        except Exception:
            pass


_install_env_fixes()

# ---------------------------------------------------------------------------
# Host preprocessing
# ---------------------------------------------------------------------------

def preprocess(x, edge_index, batch, n_graphs):
    N = x.shape[0]
    GPC = n_graphs // M
    e0 = np.asarray(edge_index[0], np.int64)
    e1 = np.asarray(edge_index[1], np.int64)
    # self-loops are excluded from the gathered edge stream (handled as a
    # cheap identity-matmul term from the SBUF-resident local z); they still
    # count toward the degrees
    src = e0
    dst = e1
    deg = (np.bincount(e1, minlength=N) + 1).astype(np.float32)
    dis = 1.0 / np.sqrt(deg)
    norm = (dis[src] * dis[dst]).astype(np.float32)
    selfnorm = (dis * dis).astype(np.float32)

    batch = np.asarray(batch, np.int64)
    owner = batch // GPC
    node_start = np.searchsorted(batch, np.arange(M) * GPC)
    node_end = np.searchsorted(batch, np.arange(M) * GPC + GPC)
    n_c = node_end - node_start
    T = int(np.ceil(n_c.max() / 128))
    T = ((T + 3) // 4) * 4  # quarter boundaries must align to whole tiles
    S = T * 128
    S4 = S // 4
    assert M * S4 <= 32767, (T, S)
    local = np.arange(N) - node_start[owner]

    eo = owner[dst]
    ld = local[dst]
    et = ld // 128
    ep = (ld % 128).astype(np.float32)
    # bucket = source node's local quarter; each quarter is AllGathered into
    # its own [M*S4, 128] table so layer k+1's descriptor generation for
    # bucket q only waits on quarter-q's collective
    ls = local[src]
    eb = ls // S4
    erel = owner[src] * S4 + (ls - eb * S4)

    # uniform (max-over-cores) padded block counts per (tile, bucket)
    key = ((eo * T + et) * 4 + eb)
    counts = np.bincount(key, minlength=M * T * 4).reshape(M, T, 4)
    nblk_tb = -(-counts.max(axis=0) // 128)  # [T, 4]

    nchunks = -(-T // CH)
    blk_of_tb = np.zeros((T, 4), np.int64)
    chunk_info = []
    call_tb = []  # (t, b) per call, in program order, for the count tensor
    gb = 0
    for k in range(nchunks):
        t0, t1 = k * CH, min((k + 1) * CH, T)
        blk0 = gb
        calls = []
        for b in range(4):
            rblk0 = gb
            for t in range(t0, t1):
                blk_of_tb[t, b] = gb
                gb += int(nblk_tb[t, b])
            nreg = gb - rblk0
            off = 0
            while off < nreg:
                nb = min(nreg - off, CALL_MAX // 128)
                calls.append((b, (rblk0 + off) * 128, nb * 128,
                              (rblk0 + off) - blk0, len(call_tb)))
                call_tb.append(None)
                off += nb
        tiles = []
        for t in range(t0, t1):
            cols = []
            for b in range(4):
                st = int(blk_of_tb[t, b] - blk0)
                cols.extend(range(st, st + int(nblk_tb[t, b])))
            tiles.append((t, cols))
        chunk_info.append(dict(blk0=blk0, nblk=gb - blk0, calls=calls,
                               tiles=tiles))
    total_blocks = gb
    total_slots = gb * 128
    ncalls = len(call_tb)

    # per-core slot arrays
    ordk = np.argsort(key, kind="stable")
    ks = key[ordk]
    group_start = np.zeros(M * T * 4 + 1, np.int64)
    np.cumsum(counts.reshape(-1), out=group_start[1:])
    pos = np.arange(len(ks)) - group_start[ks]
    slot_base = blk_of_tb.reshape(-1)
    slot = slot_base[ks % (T * 4)] * 128 + pos
    core_of = ks // (T * 4)

    er_s = erel[ordk]
    nm_s = norm[ordk]
    ep_s = ep[ordk]

    cnt_g = np.bincount(batch, minlength=n_graphs).astype(np.float32)

    in_maps = []
    for c in range(M):
        selc = core_of == c
        sl = slot[selc]
        idx16 = np.zeros(total_slots, np.int16)
        nrm = np.zeros(total_slots, np.float16)
        lds = np.zeros(total_slots, np.float16)
        idx16[sl] = er_s[selc].astype(np.int16)
        nrm[sl] = nm_s[selc].astype(np.float16)
        lds[sl] = ep_s[selc].astype(np.float16)
        idxw = np.ascontiguousarray(
            np.tile(idx16.reshape(-1, 16).T, (8, 1)))
        nrm2 = np.ascontiguousarray(nrm.reshape(-1, 128).T)
        lds2 = np.ascontiguousarray(lds.reshape(-1, 128).T)

        xT = np.zeros((128, S), np.float16)
        xT[:, :n_c[c]] = x[node_start[c]:node_end[c]].T

        gflat = np.full(T * 128, -1.0, np.float16)
        gflat[:n_c[c]] = (batch[node_start[c]:node_end[c]] - c * GPC)
        gcol = np.ascontiguousarray(gflat.reshape(T, 128).T)

        sn = np.zeros(T * 128, np.float32)
        sn[:n_c[c]] = selfnorm[node_start[c]:node_end[c]]
        snorm = np.ascontiguousarray(sn.reshape(T, 128).T)

        invcnt = np.zeros((128, 1), np.float32)
        invcnt[:GPC, 0] = 1.0 / np.maximum(cnt_g[c * GPC:(c + 1) * GPC], 1.0)

        in_maps.append(dict(idx=idxw, nrm=nrm2, lds=lds2, xT=xT, gcol=gcol,
                            invcnt=invcnt, snorm=snorm))
    meta = dict(T=T, S=S, GPC=GPC, chunks=chunk_info,
                total_blocks=total_blocks, total_slots=total_slots,
                ncalls=ncalls)
    return in_maps, meta


def make_weight_inputs(W1, b1, Wh, bh, Wout, bout):
    Wh = np.asarray(Wh, np.float32)
    iota16 = np.broadcast_to(np.arange(128, dtype=np.float16), (128, 128)).copy()
    ident = np.eye(H, dtype=np.float32)
    ident128 = np.eye(128, dtype=np.float16)
    return dict(
        ident128=ident128,
        W1=np.ascontiguousarray(W1, np.float32).astype(np.float16),
        Wh2=np.ascontiguousarray(
            Wh.transpose(1, 0, 2).reshape(H, 4 * H)).astype(np.float16),
        b14=np.ascontiguousarray(
            np.stack([b1, bh[0], bh[1], bh[2]], axis=1), np.float32),
        b5rep=np.broadcast_to(np.asarray(bh[3], np.float32), (128, H)).copy(),
        woutrep=np.broadcast_to(np.asarray(Wout, np.float32)[:, 0], (128, H)).copy(),
        iota16=iota16,
        ident=ident,
    ), dict(bout=np.asarray(bout, np.float32))


# ---------------------------------------------------------------------------
# Bass program
# ---------------------------------------------------------------------------

def build_nc(meta, weights):
    T, S = meta["T"], meta["S"]
    chunks = meta["chunks"]
    total_blocks = meta["total_blocks"]
    total_slots = meta["total_slots"]
    ncalls = meta["ncalls"]
    nc = bass.Bass("TRN2", target_bir_lowering=False)

    xT_d = nc.declare_dram_parameter("xT", [128, S], F16, isOutput=False)
    idx_d = nc.declare_dram_parameter("idx", [128, total_slots // 16], I16,
                                      isOutput=False)
    nrm_d = nc.declare_dram_parameter("nrm", [128, total_blocks], F16,
                                      isOutput=False)
    lds_d = nc.declare_dram_parameter("lds", [128, total_blocks], F16,
                                      isOutput=False)
    gcol_d = nc.declare_dram_parameter("gcol", [128, T], F16, isOutput=False)
    invcnt_d = nc.declare_dram_parameter("invcnt", [128, 1], F32, isOutput=False)
    iota16_d = nc.declare_dram_parameter("iota16", [128, 128], F16, isOutput=False)
    W1_d = nc.declare_dram_parameter("W1", [128, H], F16, isOutput=False)
    Wh2_d = nc.declare_dram_parameter("Wh2", [H, 4 * H], F16, isOutput=False)
    b14_d = nc.declare_dram_parameter("b14", [H, 4], F32, isOutput=False)
    b5rep_d = nc.declare_dram_parameter("b5rep", [128, H], F32, isOutput=False)
    woutrep_d = nc.declare_dram_parameter("woutrep", [128, H], F32, isOutput=False)
    ident_d = nc.declare_dram_parameter("ident", [H, H], F32, isOutput=False)
    ident128_d = nc.declare_dram_parameter("ident128", [128, 128], F16,
                                           isOutput=False)
    snorm_d = nc.declare_dram_parameter("snorm", [128, T], F32, isOutput=False)
    out_d = nc.declare_dram_parameter("out", [128, 1], F32, isOutput=True)
    bout = float(weights["bout"][0])

    with tile.TileContext(nc) as tc:
        with contextlib.ExitStack() as ctx:
            zfull_pool = ctx.enter_context(
                tc.tile_pool(name="zfull", bufs=1, space="DRAM"))
            dram = ctx.enter_context(tc.tile_pool(name="dram", bufs=1, space="DRAM"))
            const = ctx.enter_context(tc.tile_pool(name="const", bufs=1))
            xp = ctx.enter_context(tc.tile_pool(name="xp", bufs=3))
            gp = ctx.enter_context(tc.tile_pool(name="gp", bufs=3))
            mp = ctx.enter_context(tc.tile_pool(name="mp", bufs=3))
            hp = ctx.enter_context(tc.tile_pool(name="hp", bufs=3))
            zp = ctx.enter_context(tc.tile_pool(name="zp", bufs=3))
            pop = ctx.enter_context(tc.tile_pool(name="pop", bufs=3))
            ep = ctx.enter_context(tc.tile_pool(name="ep", bufs=1))
            ps_agg = ctx.enter_context(tc.tile_pool(name="ps_agg", bufs=2, space="PSUM"))
            ps_z = ctx.enter_context(tc.tile_pool(name="ps_z", bufs=2, space="PSUM"))
            ps_pool = ctx.enter_context(tc.tile_pool(name="ps_pool", bufs=1, space="PSUM"))

            nc.gpsimd.load_library(library_config.mlp)
            # one shared register for dma_gather's num_idxs (to_reg would
            # allocate a fresh register per call and exhaust the file)
            nidx_reg = nc.alloc_register(mybir.EngineType.Pool, "nidx_reg")

            S4 = S // 4
            zfulls = [[zfull_pool.tile([M * S4, 128], F16, addr_space="Shared",
                                       name=f"zfull{k}_{q}", tag=f"zfull{k}_{q}")
                       for q in range(4)]
                      for k in range(5)]
            bounces = [dram.tile([S, 128], F16, name=f"bounce{k}", tag=f"bounce{k}")
                       for k in range(5)]

            idx_sb = const.tile([128, total_slots // 16], I16)
            nc.sync.dma_start(idx_sb[:], idx_d[:])
            nrm_sb = const.tile([128, total_blocks], F16)
            nc.sync.dma_start(nrm_sb[:], nrm_d[:])
            lds_sb = const.tile([128, total_blocks], F16)
            nc.sync.dma_start(lds_sb[:], lds_d[:])
            gcol_sb = const.tile([128, T], F16)
            nc.sync.dma_start(gcol_sb[:], gcol_d[:])
            invcnt_sb = const.tile([128, 1], F32)
            nc.sync.dma_start(invcnt_sb[:], invcnt_d[:])
            iota16_sb = const.tile([128, 128], F16)
            nc.sync.dma_start(iota16_sb[:], iota16_d[:])
            W1_sb = const.tile([128, H], F16)
            nc.sync.dma_start(W1_sb[:], W1_d[:])
            Wh2_sb = const.tile([H, 4 * H], F16)
            nc.sync.dma_start(Wh2_sb[:], Wh2_d[:])
            b14_sb = const.tile([H, 4], F32)
            nc.sync.dma_start(b14_sb[:], b14_d[:])
            b5rep_sb = const.tile([128, H], F32)
            nc.sync.dma_start(b5rep_sb[:], b5rep_d[:])
            woutrep_sb = const.tile([128, H], F32)
            nc.sync.dma_start(woutrep_sb[:], woutrep_d[:])
            ident_sb = const.tile([H, H], F32)
            nc.sync.dma_start(ident_sb[:], ident_d[:])
            ident128_sb = const.tile([128, 128], F16)
            nc.sync.dma_start(ident128_sb[:], ident128_d[:])
            snorm_sb = const.tile([128, T], F32)
            nc.sync.dma_start(snorm_sb[:], snorm_d[:])
            # the core's own z tiles for the current layer (self-loop source)
            zall = const.tile([128, T * H], F16)

            # layer 0: z0 = x @ W1 per local tile, then AllGather
            for t in range(T):
                xt = xp.tile([128, 128], F16, tag="xt")
                nc.sync.dma_start(xt[:], xT_d[:, t * 128:(t + 1) * 128])
                pz = ps_z.tile([128, H], F32, space="PSUM", tag="pz")
                nc.tensor.matmul(out=pz[:], lhsT=xt[:], rhs=W1_sb[:],
                                 start=True, stop=True)
                zt = zp.tile([128, 128], F16, tag="zt")
                nc.scalar.copy(zt[:, :H], pz[:])
                nc.scalar.copy(zall[:, t * H:(t + 1) * H], pz[:])
                nc.sync.dma_start(bounces[0][t * 128:(t + 1) * 128, :], zt[:])
            for q in range(4):
                nc.gpsimd.collective_compute(
                    "AllGather", mybir.AluOpType.bypass,
                    replica_groups=[list(range(M))],
                    ins=[bounces[0][q * S4:(q + 1) * S4, :]],
                    outs=[zfulls[0][q][:]])

            n_pool = sum(1 for ci in chunks for (t, cols) in ci["tiles"])
            pool_ps = None
            pool_i = 0

            for layer in range(1, 6):
                zf = zfulls[layer - 1]
                if layer == 5:
                    pool_ps = ps_pool.tile([H, 128], F32, space="PSUM", tag="pool")
                for ci in chunks:
                    cb = ci["nblk"]
                    blk0 = ci["blk0"]
                    g = gp.tile([128, cb, 128], F16, tag="g")
                    for (b, slot0, nidx, blkoff, call_i) in ci["calls"]:
                        nc.gpsimd.reg_mov(nidx_reg, nidx)
                        nc.gpsimd.dma_gather(
                            g[:, blkoff:blkoff + nidx // 128, :],
                            zf[b][:],
                            idx_sb[:, slot0 // 16:(slot0 + nidx) // 16],
                            nidx, nidx_reg, 128, single_packet=False)
                    gs = g[:, :, 0:H]
                    nrm3 = nrm_sb[:, blk0:blk0 + cb, None].to_broadcast([128, cb, H])
                    nc.vector.tensor_tensor(out=gs, in0=gs, in1=nrm3,
                                            op=mybir.AluOpType.mult)
                    m01 = mp.tile([128, cb, 128], F16, tag="m01")
                    iota3 = iota16_sb[:, None, :].to_broadcast([128, cb, 128])
                    lds3 = lds_sb[:, blk0:blk0 + cb, None].to_broadcast([128, cb, 128])
                    nc.vector.tensor_tensor(out=m01[:], in0=iota3, in1=lds3,
                                            op=mybir.AluOpType.is_equal)
                    for (t, cols) in ci["tiles"]:
                        selfmsg = hp.tile([128, H], F16, tag="selfmsg")
                        nc.vector.tensor_scalar(
                            out=selfmsg[:], in0=zall[:, t * H:(t + 1) * H],
                            scalar1=snorm_sb[:, t:t + 1], scalar2=None,
                            op0=mybir.AluOpType.mult)
                        if layer < 5:
                            pT = ps_agg.tile([H, 128], F32, space="PSUM", tag="pT")
                            for j, col in enumerate(cols):
                                nc.tensor.matmul(
                                    out=pT[:], lhsT=g[:, col, 0:H],
                                    rhs=m01[:, col, :],
                                    start=(j == 0), stop=False)
                            nc.tensor.matmul(
                                out=pT[:], lhsT=selfmsg[:], rhs=ident128_sb[:],
                                start=(len(cols) == 0), stop=True)
                            hT = hp.tile([H, 128], F16, tag="hT")
                            nc.scalar.activation(
                                hT[:], pT[:], mybir.ActivationFunctionType.Relu,
                                bias=b14_sb[:, layer - 1:layer])
                            pz = ps_z.tile([128, H], F32, space="PSUM", tag="pz")
                            nc.tensor.matmul(
                                out=pz[:], lhsT=hT[:],
                                rhs=Wh2_sb[:, (layer - 1) * H:layer * H],
                                start=True, stop=True)
                            zt = zp.tile([128, 128], F16, tag="zt")
                            nc.scalar.copy(zt[:, :H], pz[:])
                            nc.scalar.copy(zall[:, t * H:(t + 1) * H], pz[:])
                            nc.sync.dma_start(
                                bounces[layer][t * 128:(t + 1) * 128, :], zt[:])
                        else:
                            p5 = ps_agg.tile([128, H], F32, space="PSUM", tag="pT")
                            for j, col in enumerate(cols):
                                nc.tensor.matmul(
                                    out=p5[:], lhsT=m01[:, col, :],
                                    rhs=g[:, col, 0:H],
                                    start=(j == 0), stop=False)
                            nc.tensor.matmul(
                                out=p5[:], lhsT=ident128_sb[:], rhs=selfmsg[:],
                                start=(len(cols) == 0), stop=True)
                            h5 = hp.tile([128, H], F16, tag="hT")
                            nc.vector.tensor_tensor(out=h5[:], in0=p5[:],
                                                    in1=b5rep_sb[:],
                                                    op=mybir.AluOpType.add)
                            nc.scalar.activation(h5[:], h5[:],
                                                 mybir.ActivationFunctionType.Relu)
                            pt = pop.tile([128, 128], F16, tag="pt")
                            nc.vector.tensor_tensor(
                                out=pt[:], in0=iota16_sb[:],
                                in1=gcol_sb[:, t:t + 1].to_broadcast([128, 128]),
                                op=mybir.AluOpType.is_equal)
                            nc.tensor.matmul(
                                out=pool_ps[:], lhsT=h5[:], rhs=pt[:],
                                start=(pool_i == 0), stop=(pool_i == n_pool - 1))
                            pool_i += 1
                if layer < 5:
                    for q in range(4):
                        nc.gpsimd.collective_compute(
                            "AllGather", mybir.AluOpType.bypass,
                            replica_groups=[list(range(M))],
                            ins=[bounces[layer][q * S4:(q + 1) * S4, :]],
                            outs=[zfulls[layer][q][:]])

            # transpose, scale by 1/cnt, layernorm, output head (f32 tail)
            pool_sb = ep.tile([H, 128], F32, tag="poolsb")
            nc.scalar.copy(pool_sb[:], pool_ps[:])
            ptr = ps_z.tile([128, H], F32, space="PSUM", tag="pz")
            nc.tensor.transpose(out=ptr[:], in_=pool_sb[:], identity=ident_sb[:])
            pooled = ep.tile([128, H], F32, tag="pooled")
            nc.vector.tensor_scalar(out=pooled[:], in0=ptr[:],
                                    scalar1=invcnt_sb[:, 0:1], scalar2=None,
                                    op0=mybir.AluOpType.mult)
            mu = ep.tile([128, 1], F32, tag="mu")
            nc.vector.tensor_reduce(out=mu[:], in_=pooled[:],
                                    axis=mybir.AxisListType.X,
                                    op=mybir.AluOpType.add)
            nc.vector.tensor_scalar(out=mu[:], in0=mu[:], scalar1=1.0 / H,
                                    scalar2=None, op0=mybir.AluOpType.mult)
            xc = ep.tile([128, H], F32, tag="xc")
            nc.vector.tensor_scalar(out=xc[:], in0=pooled[:], scalar1=mu[:, 0:1],
                                    scalar2=None, op0=mybir.AluOpType.subtract)
            sq = ep.tile([128, H], F32, tag="sq")
            nc.scalar.activation(sq[:], xc[:], mybir.ActivationFunctionType.Square)
            var = ep.tile([128, 1], F32, tag="var")
            nc.vector.tensor_reduce(out=var[:], in_=sq[:],
                                    axis=mybir.AxisListType.X,
                                    op=mybir.AluOpType.add)
            nc.vector.tensor_scalar(out=var[:], in0=var[:], scalar1=1.0 / H,
                                    scalar2=None, op0=mybir.AluOpType.mult)
            eps_col = ep.tile([128, 1], F32, tag="eps")
            nc.vector.memset(eps_col[:], 1e-5)
            std = ep.tile([128, 1], F32, tag="std")
            nc.scalar.activation(std[:], var[:], mybir.ActivationFunctionType.Sqrt,
                                 bias=eps_col[:, 0:1])
            rstd = ep.tile([128, 1], F32, tag="rstd")
            nc.vector.reciprocal(rstd[:], std[:])
            ln = ep.tile([128, H], F32, tag="ln")
            nc.vector.tensor_scalar(out=ln[:], in0=xc[:], scalar1=rstd[:, 0:1],
                                    scalar2=None, op0=mybir.AluOpType.mult)
            y = ep.tile([128, H], F32, tag="y")
            nc.vector.tensor_tensor(out=y[:], in0=ln[:], in1=woutrep_sb[:],
                                    op=mybir.AluOpType.mult)
            yr = ep.tile([128, 1], F32, tag="yr")
            nc.vector.tensor_reduce(out=yr[:], in_=y[:], axis=mybir.AxisListType.X,
                                    op=mybir.AluOpType.add)
            nc.vector.tensor_scalar(out=yr[:], in0=yr[:], scalar1=bout,
                                    scalar2=None, op0=mybir.AluOpType.add)
            nc.sync.dma_start(out_d[:], yr[:])

    from concourse.library_overlay import lower_extended_insts
    lower_extended_insts(nc)
    return nc


# ---------------------------------------------------------------------------
# Entry point
# ---------------------------------------------------------------------------

def kernel(x, edge_index, batch, W1, b1, Wh, bh, Wout, bout):
    from concourse.bass_utils import run_bass_kernel_spmd

    x = np.asarray(x, np.float32)
    edge_index = np.asarray(edge_index)
    batch = np.asarray(batch)
    n_graphs = 1000

    in_maps, meta = preprocess(x, edge_index, batch, n_graphs)
    wmaps, wmeta = make_weight_inputs(W1, b1, Wh, bh, Wout, bout)
    nc = build_nc(meta, dict(bout=wmeta["bout"]))
    for im in in_maps:
        im.update(wmaps)

    import time
    last_err = None
    for attempt in range(3):
        try:
            res = run_bass_kernel_spmd(nc, in_maps, core_ids=list(range(M)))
            break
        except Exception as e:  # transient terminal hiccups / device recovery
            last_err = e
            time.sleep(30 * (attempt + 1))
    else:
        raise last_err

    GPC = meta["GPC"]
    out = np.concatenate([res.results[c]["out"][:GPC] for c in range(M)], axis=0)
    return np.ascontiguousarray(out, np.float32)
